# revision 35
# baseline (speedup 1.0000x reference)
"""Multi-headed causal attention on 8 trn2 NeuronCores (Bass/Tile).

Sharding: tensor-parallel over heads — 2 heads per core, all 4 batches.

v2 schedule: one globally interleaved PE stream instead of
proj-phase -> attention-phase. Motivation (from the baseline trace):
the exp stream on ACT (~190us) exceeds the attention-phase PE time, so
a separate attention phase is ACT-bound; and every PE idle gap >3.4us
re-throttles the PE clock to 4/8 (HAM), which the phase transitions and
the 47us tail kept triggering.

  - Work items: P(b,q,p) = one projection accumulation group (8 matmuls
    of [128x128]@[128x512]); T(b,q) = 4 PE transposes of a vt quarter
    (deferred one P item so they never wait on the vt copy); A(b,j,m) =
    one attention m-step (row-tiled concurrent score pair -> exp on ACT
    -> AV pair, AV delayed PIPE items through a pending queue); F(b,j) =
    chunk finish (denominator bridge + reciprocal + normalize + a2a_in
    stores), triggered when the chunk's last AV pops.
  - The merger emits proj and attention streams in a global ~1:1
    PE-cycle ratio (they are nearly equal in cycles), with attention of
    batch b gated on proj progress of batch b. Attention naturally runs
    ~one batch behind proj, so ACT exp work is spread over the whole
    kernel and the PE never idles at phase boundaries.
  - Collectives: 5 AllToAlls — one per batch for b=0..2 (512KB/core),
    two half-size for b=3 — plus a tiny warmup AllToAll at t=0 to
    absorb launch skew. cats loads and out-projection micro-groups are
    injected via a ready-at-m backlog so their triggers never sit in an
    engine FIFO while a collective is still in flight. The final
    collective carries only 2 chunks, so the tail is a2a(256KB) + 16
    matmuls + bias + store.
  - PSUM: psS 2x[128,2,512]f32 (score pipeline), psC 2x[128,512]f32
    (ctx pair of the active chunk), psM 2x2KB (proj/transpose/outproj
    groups, each emitted atomically) = exactly 8 banks.
  - V is padded to [V | ones*64]: softmax denominators come out
    replicated on PSUM partitions 64..127 at zero extra PE stream time
    (out partitions are free); one ACT copy bridges them to partitions
    0..63, then reciprocal+mul on DVE.
  - DMA queues: bulk loads on the two hw DGE queues (sync/scalar),
    a2a_in stores on gpsimd, cats + out stores on sync/scalar only
    after their collective is surely done (backlog delay).
"""
import heapq
import sys

sys.path.insert(0, "/opt/trn_rl_repo")

import numpy as np

import concourse.bass as bass
import concourse.tile as tile
from concourse import bacc, mybir
from concourse.bass_utils import run_bass_kernel_spmd

B, S, D, H, HD = 4, 2048, 1024, 16, 64
NC_ = 8          # cores
PH = 2           # heads per core
SC = 512         # s_q chunk
NK = S // 128    # 16 s_k chunks of 128
ND = D // 128    # 8 contraction chunks of 128
F32 = mybir.dt.float32
BF16 = mybir.dt.bfloat16
EXP = mybir.ActivationFunctionType.Exp
GE = mybir.AluOpType.is_ge
PIPE = 3         # AV lag behind scores, in A items
# collective sets: s = 2*b + j//2, i.e. one AllToAll per half batch
# (2 chunks, 1024 tokens, 256KB/core). Small sets keep each flight short
# (~9us), resync cores often (less skew), and make the final set cheap.
NSET = 8
CATS_DELAY = 8   # A items between collective fire and cats emission.
# cats loads ride the gpsimd queue: when a fast core's cats blocks on a
# collective still in flight (cores drift ~10-15us apart), it only
# delays later a2a_in stores -- which pace to the slowest core anyway.
# Attention-critical work never sits behind them (the diagonal mask
# runs on DVE, not gpsimd).
# outproj pieces are floored into the ACT-bound late-kernel region,
# where the PE needs filler (last batch has no proj to interleave), and
# safely after their collective completed on every core.
# outproj pieces are floored into the ACT-bound last stage: attention
# there has a ~0.3us/m PE deficit (exp on ACT is the local bottleneck),
# and idle PE windows re-throttle the clock to 4/8. Filling them keeps
# the PE warm and does not delay the slowest core's finish (unlike
# holding pieces for the final drain, which queues ahead of the last
# outproj on the critical core).
OUT_FLOOR = {(0, 0): 126, (0, 1): 129, (1, 0): 132, (1, 1): 135,
             (2, 0): 138, (2, 1): 141, (3, 0): 144, (3, 1): 147,
             (4, 0): 150, (4, 1): 153}
OUT_DELAY = 3    # min A items after cats
# sets 5/6 resolve too late to floor safely; their cats+pieces run in
# the final drain
RESERVE = {(5, 0), (5, 1), (6, 0), (6, 1)}


def build():
    nc = bacc.Bacc("TRN2", target_bir_lowering=False, debug=False, num_devices=NC_)

    emb_t = nc.dram_tensor("embedded_t", [B, 4, 128, ND, SC], BF16,
                           kind="ExternalInput").ap()
    w_qkv = nc.dram_tensor("w_qkv", [128, 3072], BF16, kind="ExternalInput").ap()
    wo_t = nc.dram_tensor("wo_t", [ND, 128, D], BF16, kind="ExternalInput").ap()
    bo_row = nc.dram_tensor("bo_row", [1, D], F32, kind="ExternalInput").ap()
    out_shard = nc.dram_tensor("out_shard", [1024, D], F32, kind="ExternalOutput").ap()

    with tile.TileContext(nc) as tc:
        _build_body(nc, tc, emb_t, w_qkv, wo_t, bo_row, out_shard)

    nc.compile()
    return nc


def _build_body(nc, tc, emb_t, w_qkv, wo_t, bo_row, out_shard):
    from contextlib import ExitStack

    ctx = ExitStack()
    with ctx:
        const = ctx.enter_context(tc.tile_pool(name="const", bufs=1))
        psS = ctx.enter_context(tc.tile_pool(name="psS", bufs=2, space="PSUM"))
        psC = ctx.enter_context(tc.tile_pool(name="psC", bufs=1, space="PSUM"))
        psM = ctx.enter_context(tc.tile_pool(name="psM", bufs=2, space="PSUM"))
        dram = ctx.enter_context(tc.tile_pool(name="dram", bufs=1, space="DRAM"))

        etp = ctx.enter_context(tc.tile_pool(name="etp", bufs=4))
        et0p = ctx.enter_context(tc.tile_pool(name="et0p", bufs=1))
        qtp = ctx.enter_context(tc.tile_pool(name="qtp", bufs=4))
        ktp = ctx.enter_context(tc.tile_pool(name="ktp", bufs=4))
        vtp = ctx.enter_context(tc.tile_pool(name="vtp", bufs=2))
        vsb = ctx.enter_context(tc.tile_pool(name="vsb", bufs=4))
        exp_p = ctx.enter_context(tc.tile_pool(name="exp_p", bufs=6))
        rc_p = ctx.enter_context(tc.tile_pool(name="rc_p", bufs=3))
        cn_p = ctx.enter_context(tc.tile_pool(name="cn_p", bufs=6))
        cat_p = ctx.enter_context(tc.tile_pool(name="cat_p", bufs=3))
        ob_p = ctx.enter_context(tc.tile_pool(name="ob_p", bufs=3))

        # ---- warmup collective: absorb launch skew ----
        warm_in = dram.tile([NC_, 1, 16], BF16, tag="warm_in", name="warm_in")
        warm_out = dram.tile([NC_, 1, 16], BF16, tag="warm_out", name="warm_out")
        nc.gpsimd.collective_compute(
            "AllToAll", mybir.AluOpType.bypass,
            replica_groups=[list(range(NC_))],
            ins=[warm_in.opt()], outs=[warm_out.opt()])

        # ---- HAM pre-warm: the PE is idle from engine boot until the
        # first DMAs land (~15us), and would start at the cold 4/8
        # clock. Dummy matmuls (no deps, no consumers; start=True
        # overwrites the psum later) trip the activity monitor so the
        # real projection starts at full clock. They queue behind the
        # warmup collective's gpsimd rendezvous, which times their burst
        # to end right as the first weight/activation DMAs complete.
        dummy = const.tile([128, 128], BF16, tag="dummy")
        nc.gpsimd.memset(dummy[:], 0.5)
        warm_ps = psM.tile([128, SC], F32, tag="M", name="warm_ps")
        for i in range(48):
            nc.tensor.matmul(warm_ps[:, 0:128], lhsT=dummy[:], rhs=dummy[:],
                             start=True, stop=True)
        # pre-load the EXP activation table while ACT is idle, so the
        # first real exp doesn't pay the 1.3us table load
        dume = rc_p.tile([64, PH, SC], F32, tag="dn", name="dume")
        nc.scalar.activation(out=dume[:, 0, 0:128], in_=dummy[0:64, :],
                             func=EXP, scale=0.125)

        # ---- startup DMAs: only what the first matmuls need, first ----
        wq_all = const.tile([128, 24, 128], BF16, tag="wq_all")
        nc.sync.dma_start(out=wq_all[:, 0:8, :], in_=w_qkv[:, 0:1024])
        wq_sb = [[wq_all[:, 8 * p + c, :] for c in range(ND)] for p in range(3)]

        # batch-0 j4=0: two small tiles for a fast first matmul, the rest
        # as one slab (few dma_starts: each queue only holds ~4 in its
        # ring; excess triggers block the issuing engine's FIFO)
        et0 = {}
        for c in range(2):
            t = et0p.tile([128, SC], BF16, tag=f"et0_{c}", name=f"et0_{c}")
            nc.scalar.dma_start(out=t[:], in_=emb_t[0, 0, :, c, :])
            et0[c] = t
        et0b = et0p.tile([128, ND - 2, SC], BF16, tag="et0b", name="et0b")
        nc.scalar.dma_start(out=et0b[:], in_=emb_t[0, 0, :, 2:ND, :])
        nc.sync.dma_start(out=wq_all[:, 8:24, :], in_=w_qkv[:, 1024:3072])

        quarters = {}

        def fetch_quarter(b, q):
            t = etp.tile([128, ND, SC], BF16, tag="eth", name=f"etq{b}_{q}")
            for k, eng in enumerate((nc.sync, nc.scalar)):
                eng.dma_start(out=t[:, 4 * k:4 * k + 4, :],
                              in_=emb_t[b, q, :, 4 * k:4 * k + 4, :])
            quarters[(b, q)] = t

        def et_ap(b, j4, c):
            if b == 0 and j4 == 0:
                return et0[c][:] if c < 2 else et0b[:, c - 2, :]
            return quarters[(b, j4)][:, c, :]

        fetch_quarter(0, 1)

        bo_sb = const.tile([1, D], F32, tag="bo1")
        nc.sync.dma_start(out=bo_sb[:], in_=bo_row[:])
        bo_b = const.tile([128, D], F32, tag="bob")
        nc.gpsimd.partition_broadcast(bo_b[:], bo_sb[:])

        ident = const.tile([128, 128], BF16, tag="ident")
        nc.gpsimd.memset(ident[:], 1.0)
        nc.gpsimd.affine_select(out=ident[:], in_=ident[:], compare_op=GE,
                                fill=0.0, base=0, pattern=[[-1, 128]],
                                channel_multiplier=1)
        nc.gpsimd.affine_select(out=ident[:], in_=ident[:], compare_op=GE,
                                fill=0.0, base=0, pattern=[[1, 128]],
                                channel_multiplier=-1)

        # causal mask for diagonal 128-blocks: cmask[k, 0, q] = (q >= k);
        # applied on DVE so cats loads on gpsimd never gate the AV chain
        cmask = const.tile([128, 1, 128], BF16, tag="cmask")
        nc.gpsimd.memset(cmask[:], 1.0)
        nc.gpsimd.affine_select(out=cmask[:], in_=cmask[:], compare_op=GE,
                                fill=0.0, base=0, pattern=[[0, 1], [1, 128]],
                                channel_multiplier=-1)

        wot_sb = [const.tile([128, D], BF16, tag=f"wo{c}", name=f"wo{c}")
                  for c in range(ND)]

        a2a_in = [dram.tile([NC_, 128, 128], BF16, tag=f"a2a_in{s}",
                            name=f"a2a_in{s}") for s in range(NSET)]
        a2a_out = [dram.tile([NC_, 128, 128], BF16, tag=f"a2a_out{s}",
                             name=f"a2a_out{s}") for s in range(NSET)]

        # ---- per-batch persistent tiles ----
        qt, kt, vt, v01 = {}, {}, {}, {}

        def open_batch(b):
            qt[b] = qtp.tile([128, S], BF16, tag="qt", name=f"qt{b}")
            kt[b] = ktp.tile([128, S], BF16, tag="kt", name=f"kt{b}")
            vt[b] = vtp.tile([128, S], BF16, tag="vt", name=f"vt{b}")
            v01[b] = [vsb.tile([128, NK, 128], BF16, tag=f"v{h}",
                               name=f"v{b}_{h}") for h in range(PH)]
            for h in range(PH):
                nc.vector.memset(v01[b][h][:, :, 64:128], 1.0)

        # ---- work item emitters ----
        wot_loaded = [0]
        pf_idx = [2]   # quarters 0 (et0) and 1 already fetched

        def emit_P(b, q, p):
            if p == 0:
                if b == 0 and q == 0:
                    open_batch(0)
                i = pf_idx[0]
                if i < 16:
                    fetch_quarter(i // 4, i % 4)
                    pf_idx[0] += 1
                if wot_loaded[0] < ND and b >= 1:
                    # wo isn't needed until the first outproj floor
                    # (~m=118); keep it off the early scalar queue so
                    # the exp stream never sits behind its triggers
                    c = wot_loaded[0]
                    nc.sync.dma_start(out=wot_sb[c][:], in_=wo_t[c])
                    wot_loaded[0] += 1
            ps = psM.tile([128, SC], F32, tag="M", name=f"pj{b}_{q}_{p}")
            for c in range(ND):
                nc.tensor.matmul(
                    ps[:], lhsT=wq_sb[p][c], rhs=et_ap(b, q, c),
                    start=(c == 0), stop=(c == ND - 1))
            sl = slice(SC * q, SC * (q + 1))
            if p == 0:
                nc.vector.tensor_copy(qt[b][:, sl], ps[:])
            elif p == 1:
                nc.vector.tensor_copy(kt[b][:, sl], ps[:])
            else:
                nc.vector.tensor_copy(vt[b][:, sl], ps[:])
                if q == 3 and b + 1 < B:
                    open_batch(b + 1)

        def emit_T(b, g4):
            pt = psM.tile([128, 4, 128], BF16, tag="M", name=f"tr{b}_{g4}")
            for i in range(4):
                sk = 4 * g4 + i
                nc.tensor.transpose(pt[:, i, :],
                                    vt[b][:, 128 * sk:128 * (sk + 1)],
                                    ident[:])
            for h in range(PH):
                nc.vector.tensor_copy(
                    v01[b][h][:, 4 * g4:4 * (g4 + 1), 0:64],
                    pt[:, :, 64 * h:64 * (h + 1)])

        # ---- attention machinery ----
        pending = []
        backlog = []      # heap of (ready_m, seq, thunk)
        bseq = [0]
        tail_backlog = []  # thunks drained only after the last A item
        m_count = [0]

        def backlog_push(ready, thunk):
            heapq.heappush(backlog, (ready, bseq[0], thunk))
            bseq[0] += 1
        ctx_ps = {}
        set_left = {s: 2 for s in range(NSET)}

        def emit_A(b, j, m):
            c0 = max(0, 128 * m - SC * j)
            psc = psS.tile([128, PH, SC], F32, tag="S", name=f"sc{b}_{j}_{m}")
            for h in range(PH):
                nc.tensor.matmul(
                    psc[:, h, c0:SC],
                    lhsT=kt[b][64 * h:64 * (h + 1), 128 * m:128 * (m + 1)],
                    rhs=qt[b][64 * h:64 * (h + 1), SC * j + c0:SC * (j + 1)],
                    start=True, stop=True)
            ex = exp_p.tile([128, PH, SC], BF16, tag="ex",
                            name=f"ex{b}_{j}_{m}")
            nc.scalar.activation(out=ex[:, :, c0:], in_=psc[:, :, c0:],
                                 func=EXP, scale=0.125)
            if m >= 4 * j:  # diagonal tile: zero k>q entries in the 128 block
                ea = ex[:, :, c0:c0 + 128]
                cm, eb = bass.broadcast_tensor_aps(cmask[:], ea)
                nc.vector.tensor_mul(ea, eb, cm)
            pending.append((b, j, m, ex, m == 4 * j + 3))
            m_count[0] += 1
            if len(pending) > PIPE:
                pop_av()

        def pop_av():
            b, j, m, ex, is_last = pending.pop(0)
            if m == 0:
                ctx_ps[(b, j)] = psC.tile([128, PH, SC], F32, tag="C",
                                          name=f"ctx{b}_{j}")
            cp = ctx_ps[(b, j)]
            c0 = max(0, 128 * m - SC * j)
            for h in range(PH):
                nc.tensor.matmul(
                    cp[:, h, c0:SC], lhsT=v01[b][h][:, m, :],
                    rhs=ex[:, h, c0:SC],
                    start=(m == 0), stop=is_last)
            if is_last:
                finish_chunk(b, j, cp)
                del ctx_ps[(b, j)]

        def finish_chunk(b, j, cp):
            s = 2 * b + j // 2
            # bridge replicated denominators to partitions 0..63 (ACT is
            # the only engine that can shift partitions out of PSUM)
            dn = rc_p.tile([64, PH, SC], F32, tag="dn")
            nc.scalar.copy(dn[:], cp[64:128, :, :])
            rc = rc_p.tile([64, PH, SC], F32, tag="rc")
            nc.vector.reciprocal_approx_fast(rc[:], dn[:])
            cn = cn_p.tile([64, PH, SC], BF16, tag="cn")
            nc.vector.tensor_mul(cn[:], cp[0:64, :, :], rc[:])
            for h in range(PH):
                for f in range(4):
                    # split the last batch's stores across two queues so
                    # the final pre-collective chain is short; earlier
                    # batches keep sync free for prefetches
                    eng = nc.gpsimd if (f < 2 or b < 3) else nc.sync
                    eng.dma_start(
                        out=a2a_in[s][4 * (j % 2) + f,
                                      64 * h:64 * (h + 1), :],
                        in_=cn[:, h, 128 * f:128 * (f + 1)])
            set_left[s] -= 1
            if set_left[s] == 0:
                fire_set(s)

        def fire_set(s):
            nc.gpsimd.collective_compute(
                "AllToAll", mybir.AluOpType.bypass,
                replica_groups=[list(range(NC_))],
                ins=[a2a_in[s].opt()], outs=[a2a_out[s].opt()])
            if s in (5, 6):
                # run these cats in the final drain, after the last
                # chunk's a2a_in stores and the last collective's fire
                # have been emitted: their gpsimd-blocking wait can then
                # never delay the final collective
                tail_backlog.append(_mk_cats(s))
            elif s == NSET - 1:
                backlog_push(0, _mk_cats(s))
            # cats(s-2) now: collective s-2 completed long ago (two full
            # collective periods), so its trigger never blocks the
            # gpsimd queue -- a slow collective then cannot convoy into
            # delayed a2a stores for the following sets
            if 0 <= s - 2 <= 4:
                backlog_push(0, _mk_cats(s - 2))

        def _mk_cats(s):
            def thunk():
                cats = []
                # last set: 3 parallel queues so the loads (and the
                # final outproj's first weights) land ~1us after the
                # collective completes; tail sets 5/6: sync+gpsimd (both
                # free in the drain); earlier sets: gpsimd only (a
                # blocked trigger there only delays later a2a stores)
                if s == NSET - 1:
                    engs = (nc.sync, nc.scalar, nc.gpsimd)
                elif s in (5, 6):
                    engs = (nc.sync, nc.gpsimd)
                else:
                    engs = (nc.gpsimd,)
                for r in range(NC_):
                    ct = cat_p.tile([128, 128], BF16, tag=f"cat{r}",
                                    name=f"cat{s}_{r}")
                    engs[r % len(engs)].dma_start(out=ct[:], in_=a2a_out[s][r])
                    cats.append(ct)
                for n in range(2):
                    if (s, n) in RESERVE:
                        tail_backlog.append(_mk_outproj(s, n, cats))
                    else:
                        rdy = max(m_count[0] + OUT_DELAY + 2 * n,
                                  OUT_FLOOR.get((s, n), 0))
                        backlog_push(rdy, _mk_outproj(s, n, cats))
            return thunk

        def _mk_outproj(s, n, cats):
            def thunk():
                po = psM.tile([128, SC], F32, tag="M", name=f"po{s}_{n}")
                for kp in range(ND):
                    nc.tensor.matmul(
                        po[:],
                        lhsT=cats[kp][:],
                        rhs=wot_sb[kp][:, SC * n:SC * (n + 1)],
                        start=(kp == 0), stop=(kp == ND - 1))
                ob = ob_p.tile([128, SC], F32, tag="ob")
                nc.vector.tensor_add(ob[:], po[:],
                                     bo_b[:, SC * n:SC * (n + 1)])
                r0 = 128 * s
                eng = nc.scalar if (s == NSET - 1 and n == 1) else nc.sync
                eng.dma_start(
                    out=out_shard[r0:r0 + 128, SC * n:SC * (n + 1)],
                    in_=ob[:])
            return thunk

        # ---- the merger: one global interleaved stream ----
        projW = []
        for b in range(B):
            for q in range(4):
                projW.append(("P", b, q, 0))
                if q > 0:
                    projW.append(("T", b, q - 1))
                elif b > 0:
                    projW.append(("T", b - 1, 3))
                projW.append(("P", b, q, 1))
                projW.append(("P", b, q, 2))
        projW.append(("T", 3, 3))
        attnW = [("A", b, j, m)
                 for b in range(B) for j in range(4) for m in range(4 * j + 4)]

        def cost(it):
            if it[0] == "P":
                return 4500
            if it[0] == "T":
                return 700
            _, b, j, m = it
            return 3 * (SC - max(0, 128 * m - SC * j)) + 400

        TP = sum(cost(it) for it in projW)
        TA = sum(cost(it) for it in attnW)
        # proj stream tracks attention progress plus a small lead; the
        # per-quarter readiness gate then keeps attention just behind
        # proj, so proj finishes as late as possible and the ACT-bound
        # post-proj stretch (exp is the local bottleneck) stays short
        LEAD = 0.10
        emitted = set()
        pi = ai = 0
        cp_c = ca_c = 0

        def attn_ready():
            if ai >= len(attnW):
                return False
            _, b, j, m = attnW[ai]
            qn = max(j, m // 4)
            if ("P", b, qn, 2) not in emitted:
                return False
            return ("T", b, m // 4) in emitted

        def emit_item(it):
            emitted.add(it)
            if it[0] == "P":
                emit_P(it[1], it[2], it[3])
            elif it[0] == "T":
                emit_T(it[1], it[2])
            else:
                emit_A(it[1], it[2], it[3])

        while pi < len(projW) or ai < len(attnW):
            if backlog and backlog[0][0] <= m_count[0]:
                heapq.heappop(backlog)[2]()
                continue
            ready = attn_ready()
            if pi < len(projW) and (
                    not ready or cp_c / TP < ca_c / TA + LEAD):
                cp_c += cost(projW[pi])
                emit_item(projW[pi])
                pi += 1
            elif ready:
                ca_c += cost(attnW[ai])
                emit_item(attnW[ai])
                ai += 1
            else:
                # attention gated and proj exhausted: drain backlog
                if backlog:
                    heapq.heappop(backlog)[2]()
                else:
                    raise RuntimeError("scheduler stuck")

        while pending:        # final AVs; fires the last collective
            pop_av()
        ti = 0                # reserved pieces + tail cats: PE food
        while ti < len(tail_backlog):   # (grows while iterating)
            tail_backlog[ti]()
            ti += 1
        while backlog:        # last cats + out-projection
            heapq.heappop(backlog)[2]()


_NC_CACHE = None


def _get_nc():
    global _NC_CACHE
    if _NC_CACHE is None:
        _NC_CACHE = build()
    return _NC_CACHE


def kernel(embedded, Wq, Wk, Wv, Wo, bo, _trace=False):
    import ml_dtypes
    embedded = np.asarray(embedded, np.float32)
    # emb_r[b, q, p, c, s'] = embedded[b, 512q + s', 128c + p]
    emb_r = np.ascontiguousarray(
        embedded.reshape(B, 4, SC, ND, 128).transpose(0, 1, 4, 3, 2)
    ).astype(ml_dtypes.bfloat16)
    W = np.stack([np.asarray(Wq), np.asarray(Wk), np.asarray(Wv)]).astype(
        np.float32)
    wo_t = np.ascontiguousarray(np.asarray(Wo, np.float32).T).astype(
        ml_dtypes.bfloat16).reshape(ND, 128, D)
    bo_row = np.asarray(bo, np.float32).reshape(1, D)

    in_maps = []
    for c in range(NC_):
        w = W[:, 2 * c:2 * c + 2]                  # [3, 2, D, HD]
        w = np.ascontiguousarray(w.transpose(0, 2, 1, 3)).reshape(
            3, ND, 128, 128)
        # contiguous SBUF image: [p, 3*ND chunks, 128] -> [128, 3072]
        w = np.ascontiguousarray(w.transpose(2, 0, 1, 3)).reshape(
            128, 3072).astype(ml_dtypes.bfloat16)
        in_maps.append({
            "embedded_t": emb_r,
            "w_qkv": w,
            "wo_t": wo_t,
            "bo_row": bo_row,
        })

    nc = _get_nc()
    res = run_bass_kernel_spmd(nc, in_maps, core_ids=list(range(NC_)),
                               trace=_trace)

    out = np.empty((B, S, D), np.float32)
    for c in range(NC_):
        r = res.results[c]["out_shard"]            # [1024, D]
        for s in range(8):
            b, half = s // 2, s % 2
            out[b, 1024 * half + 128 * c:1024 * half + 128 * c + 128, :] = \
                r[128 * s:128 * s + 128]
    if _trace:
        return out, res
    return out


# revision 37
# speedup vs baseline: 1.0110x; 1.0110x over previous
"""Multi-headed causal attention on 8 trn2 NeuronCores (Bass/Tile).

Sharding: tensor-parallel over heads — 2 heads per core, all 4 batches.
~335us median (baseline 421us). Design is built around three measured
facts: (1) the exp stream on ACT (~175us) exceeds attention's own PE
time, so any attention-only phase is ACT-bound; (2) every PE idle gap
>3.4us re-throttles the PE clock to 4/8 (HAM) and costs double; (3)
cores drift 10-15us apart (per-chip power throttle), so exec time is
the SLOWEST core's path, and any engine-FIFO wait on a collective can
cascade (a blocked trigger delays that queue's later a2a stores, which
delays the next collective for everyone).

  - One globally interleaved PE stream. Work items: P(b,q,p) = one
    projection accumulation group (8 matmuls [128x128]@[128x512]);
    T(b,q) = 4 PE transposes of a vt quarter (deferred one P item so
    they never wait on the vt copy); A(b,j,m) = one attention m-step
    (row-tiled concurrent score pair for the 2 heads -> exp on ACT ->
    2 AV matmuls, AVs delayed PIPE items through a pending queue);
    F(b,j) = chunk finish, triggered when the chunk's last AV pops.
  - The merger keeps proj a small LEAD ahead of attention progress with
    per-quarter readiness gating: attention runs just behind proj, exp
    work is spread over the whole kernel, and proj finishes as late as
    possible so the ACT-bound post-proj stretch stays short. The
    remaining stretch deficit is filled by floored outproj pieces.
  - Collectives: 8 half-batch AllToAlls (s = 2b + j//2, 256KB/core,
    ~4-9us data each) + a tiny warmup AllToAll at t=0. Many small sets
    resync the drifting cores often and make the final set cheap.
  - cats loads ride the gpsimd queue and are emitted only when set s+2
    fires (collective s is then long complete -> the trigger never
    actually blocks; nothing attention-critical lives on gpsimd - the
    diagonal causal mask is a DVE multiply by a precomputed mask tile).
    Sets 5/6 cats + pieces run in the final drain, after the last
    chunk's stores and the final collective's fire are emitted. The
    last set's cats split across sync/scalar/gpsimd so the final
    outproj starts ~1us after its collective completes.
  - PSUM: psS 2x[128,2,512]f32 (score pipeline), psC 1x[128,2,512]f32
    (ctx+denominators of the active chunk, single-AP finish ops), psM
    2x2KB (proj/transpose/outproj groups, each emitted atomically) =
    exactly 8 banks.
  - V is padded to [V | ones*64]: softmax denominators come out
    replicated on PSUM partitions 64..127 at zero extra PE stream time
    (out partitions are free); one ACT copy bridges them to partitions
    0..63, then reciprocal+mul on DVE.
  - Startup: ~12us of framework preamble is fixed; few large dma_starts
    (each hw queue ring holds only ~4 - excess triggers block the
    issuing engine's FIFO, which once delayed the first exp by 10us);
    dummy matmuls pre-warm the HAM clock gate while DMAs land, and a
    dummy activation pre-loads the EXP table.
  - DMA queues: bulk loads on the two hw DGE queues (sync/scalar),
    a2a_in stores on gpsimd (last batch: split gpsimd+sync), out stores
    on sync (last piece scalar).
"""
import heapq
import sys

sys.path.insert(0, "/opt/trn_rl_repo")

import numpy as np

import concourse.bass as bass
import concourse.tile as tile
from concourse import bacc, mybir
from concourse.bass_utils import run_bass_kernel_spmd

B, S, D, H, HD = 4, 2048, 1024, 16, 64
NC_ = 8          # cores
PH = 2           # heads per core
SC = 512         # s_q chunk
NK = S // 128    # 16 s_k chunks of 128
ND = D // 128    # 8 contraction chunks of 128
F32 = mybir.dt.float32
BF16 = mybir.dt.bfloat16
EXP = mybir.ActivationFunctionType.Exp
GE = mybir.AluOpType.is_ge
PIPE = 3         # AV lag behind scores, in A items
# collective sets: s = 2*b + j//2, i.e. one AllToAll per half batch
# (2 chunks, 1024 tokens, 256KB/core). Small sets keep each flight short
# (~9us), resync cores often (less skew), and make the final set cheap.
NSET = 8
# outproj pieces are floored into the ACT-bound last stage: attention
# there has a ~0.3us/m PE deficit (exp on ACT is the local bottleneck),
# and idle PE windows re-throttle the clock to 4/8. Filling them keeps
# the PE warm and does not delay the slowest core's finish (unlike
# holding pieces for the final drain, which queues ahead of the last
# outproj on the critical core).
OUT_FLOOR = {(0, 0): 126, (0, 1): 129, (1, 0): 132, (1, 1): 135,
             (2, 0): 138, (2, 1): 141, (3, 0): 144, (3, 1): 147,
             (4, 0): 150, (4, 1): 153}
OUT_DELAY = 3    # min A items after cats
# sets 5/6 resolve too late to floor safely; their cats+pieces run in
# the final drain
RESERVE = {(5, 0), (5, 1), (6, 0), (6, 1)}


def build():
    nc = bacc.Bacc("TRN2", target_bir_lowering=False, debug=False, num_devices=NC_)

    emb_t = nc.dram_tensor("embedded_t", [B, 4, 128, ND, SC], BF16,
                           kind="ExternalInput").ap()
    w_qkv = nc.dram_tensor("w_qkv", [128, 3072], BF16, kind="ExternalInput").ap()
    wo_t = nc.dram_tensor("wo_t", [ND, 128, D], BF16, kind="ExternalInput").ap()
    bo_row = nc.dram_tensor("bo_row", [1, D], F32, kind="ExternalInput").ap()
    out_shard = nc.dram_tensor("out_shard", [1024, D], F32, kind="ExternalOutput").ap()

    with tile.TileContext(nc) as tc:
        _build_body(nc, tc, emb_t, w_qkv, wo_t, bo_row, out_shard)

    nc.compile()
    return nc


def _build_body(nc, tc, emb_t, w_qkv, wo_t, bo_row, out_shard):
    from contextlib import ExitStack

    ctx = ExitStack()
    with ctx:
        const = ctx.enter_context(tc.tile_pool(name="const", bufs=1))
        psS = ctx.enter_context(tc.tile_pool(name="psS", bufs=2, space="PSUM"))
        psC = ctx.enter_context(tc.tile_pool(name="psC", bufs=1, space="PSUM"))
        psM = ctx.enter_context(tc.tile_pool(name="psM", bufs=2, space="PSUM"))
        dram = ctx.enter_context(tc.tile_pool(name="dram", bufs=1, space="DRAM"))

        etp = ctx.enter_context(tc.tile_pool(name="etp", bufs=4))
        et0p = ctx.enter_context(tc.tile_pool(name="et0p", bufs=1))
        qtp = ctx.enter_context(tc.tile_pool(name="qtp", bufs=4))
        ktp = ctx.enter_context(tc.tile_pool(name="ktp", bufs=4))
        vtp = ctx.enter_context(tc.tile_pool(name="vtp", bufs=2))
        vsb = ctx.enter_context(tc.tile_pool(name="vsb", bufs=4))
        exp_p = ctx.enter_context(tc.tile_pool(name="exp_p", bufs=6))
        rc_p = ctx.enter_context(tc.tile_pool(name="rc_p", bufs=3))
        cn_p = ctx.enter_context(tc.tile_pool(name="cn_p", bufs=6))
        cat_p = ctx.enter_context(tc.tile_pool(name="cat_p", bufs=3))
        ob_p = ctx.enter_context(tc.tile_pool(name="ob_p", bufs=3))

        # ---- warmup collective: absorb launch skew ----
        warm_in = dram.tile([NC_, 1, 16], BF16, tag="warm_in", name="warm_in")
        warm_out = dram.tile([NC_, 1, 16], BF16, tag="warm_out", name="warm_out")
        nc.gpsimd.collective_compute(
            "AllToAll", mybir.AluOpType.bypass,
            replica_groups=[list(range(NC_))],
            ins=[warm_in.opt()], outs=[warm_out.opt()])

        # ---- HAM pre-warm: the PE is idle from engine boot until the
        # first DMAs land (~15us), and would start at the cold 4/8
        # clock. Dummy matmuls (no deps, no consumers; start=True
        # overwrites the psum later) trip the activity monitor so the
        # real projection starts at full clock. They queue behind the
        # warmup collective's gpsimd rendezvous, which times their burst
        # to end right as the first weight/activation DMAs complete.
        dummy = const.tile([128, 128], BF16, tag="dummy")
        nc.gpsimd.memset(dummy[:], 0.5)
        warm_ps = psM.tile([128, SC], F32, tag="M", name="warm_ps")
        for i in range(48):
            nc.tensor.matmul(warm_ps[:, 0:128], lhsT=dummy[:], rhs=dummy[:],
                             start=True, stop=True)
        # pre-load the EXP activation table while ACT is idle, so the
        # first real exp doesn't pay the 1.3us table load
        dume = rc_p.tile([64, PH, SC], F32, tag="dn", name="dume")
        nc.scalar.activation(out=dume[:, 0, 0:128], in_=dummy[0:64, :],
                             func=EXP, scale=0.125)

        # ---- startup DMAs: only what the first matmuls need, first ----
        wq_all = const.tile([128, 24, 128], BF16, tag="wq_all")
        nc.sync.dma_start(out=wq_all[:, 0:8, :], in_=w_qkv[:, 0:1024])
        wq_sb = [[wq_all[:, 8 * p + c, :] for c in range(ND)] for p in range(3)]

        # batch-0 j4=0: two small tiles for a fast first matmul, the rest
        # as one slab (few dma_starts: each queue only holds ~4 in its
        # ring; excess triggers block the issuing engine's FIFO)
        et0 = {}
        for c in range(2):
            t = et0p.tile([128, SC], BF16, tag=f"et0_{c}", name=f"et0_{c}")
            nc.scalar.dma_start(out=t[:], in_=emb_t[0, 0, :, c, :])
            et0[c] = t
        et0b = et0p.tile([128, ND - 2, SC], BF16, tag="et0b", name="et0b")
        nc.scalar.dma_start(out=et0b[:], in_=emb_t[0, 0, :, 2:ND, :])
        nc.sync.dma_start(out=wq_all[:, 8:24, :], in_=w_qkv[:, 1024:3072])

        quarters = {}

        def fetch_quarter(b, q):
            t = etp.tile([128, ND, SC], BF16, tag="eth", name=f"etq{b}_{q}")
            for k, eng in enumerate((nc.sync, nc.scalar)):
                eng.dma_start(out=t[:, 4 * k:4 * k + 4, :],
                              in_=emb_t[b, q, :, 4 * k:4 * k + 4, :])
            quarters[(b, q)] = t

        def et_ap(b, j4, c):
            if b == 0 and j4 == 0:
                return et0[c][:] if c < 2 else et0b[:, c - 2, :]
            return quarters[(b, j4)][:, c, :]

        fetch_quarter(0, 1)

        bo_sb = const.tile([1, D], F32, tag="bo1")
        nc.sync.dma_start(out=bo_sb[:], in_=bo_row[:])
        bo_b = const.tile([128, D], F32, tag="bob")
        nc.gpsimd.partition_broadcast(bo_b[:], bo_sb[:])

        ident = const.tile([128, 128], BF16, tag="ident")
        nc.gpsimd.memset(ident[:], 1.0)
        nc.gpsimd.affine_select(out=ident[:], in_=ident[:], compare_op=GE,
                                fill=0.0, base=0, pattern=[[-1, 128]],
                                channel_multiplier=1)
        nc.gpsimd.affine_select(out=ident[:], in_=ident[:], compare_op=GE,
                                fill=0.0, base=0, pattern=[[1, 128]],
                                channel_multiplier=-1)

        # causal mask for diagonal 128-blocks: cmask[k, 0, q] = (q >= k);
        # applied on DVE so cats loads on gpsimd never gate the AV chain
        cmask = const.tile([128, 1, 128], BF16, tag="cmask")
        nc.gpsimd.memset(cmask[:], 1.0)
        nc.gpsimd.affine_select(out=cmask[:], in_=cmask[:], compare_op=GE,
                                fill=0.0, base=0, pattern=[[0, 1], [1, 128]],
                                channel_multiplier=-1)

        wot_sb = [const.tile([128, D], BF16, tag=f"wo{c}", name=f"wo{c}")
                  for c in range(ND)]

        a2a_in = [dram.tile([NC_, 128, 128], BF16, tag=f"a2a_in{s}",
                            name=f"a2a_in{s}") for s in range(NSET)]
        a2a_out = [dram.tile([NC_, 128, 128], BF16, tag=f"a2a_out{s}",
                             name=f"a2a_out{s}") for s in range(NSET)]

        # ---- per-batch persistent tiles ----
        qt, kt, vt, v01 = {}, {}, {}, {}

        def open_batch(b):
            qt[b] = qtp.tile([128, S], BF16, tag="qt", name=f"qt{b}")
            kt[b] = ktp.tile([128, S], BF16, tag="kt", name=f"kt{b}")
            vt[b] = vtp.tile([128, S], BF16, tag="vt", name=f"vt{b}")
            v01[b] = [vsb.tile([128, NK, 128], BF16, tag=f"v{h}",
                               name=f"v{b}_{h}") for h in range(PH)]
            for h in range(PH):
                nc.vector.memset(v01[b][h][:, :, 64:128], 1.0)

        # ---- work item emitters ----
        wot_loaded = [0]
        pf_idx = [2]   # quarters 0 (et0) and 1 already fetched

        def emit_P(b, q, p):
            if p == 0:
                if b == 0 and q == 0:
                    open_batch(0)
                i = pf_idx[0]
                if i < 16:
                    fetch_quarter(i // 4, i % 4)
                    pf_idx[0] += 1
                if wot_loaded[0] < ND and b >= 1:
                    # wo isn't needed until the first outproj floor
                    # (~m=118); keep it off the early scalar queue so
                    # the exp stream never sits behind its triggers
                    c = wot_loaded[0]
                    nc.sync.dma_start(out=wot_sb[c][:], in_=wo_t[c])
                    wot_loaded[0] += 1
            ps = psM.tile([128, SC], F32, tag="M", name=f"pj{b}_{q}_{p}")
            for c in range(ND):
                nc.tensor.matmul(
                    ps[:], lhsT=wq_sb[p][c], rhs=et_ap(b, q, c),
                    start=(c == 0), stop=(c == ND - 1))
            sl = slice(SC * q, SC * (q + 1))
            if p == 0:
                nc.vector.tensor_copy(qt[b][:, sl], ps[:])
            elif p == 1:
                nc.vector.tensor_copy(kt[b][:, sl], ps[:])
            else:
                nc.vector.tensor_copy(vt[b][:, sl], ps[:])
                if q == 3 and b + 1 < B:
                    open_batch(b + 1)

        def emit_T(b, g4):
            pt = psM.tile([128, 4, 128], BF16, tag="M", name=f"tr{b}_{g4}")
            for i in range(4):
                sk = 4 * g4 + i
                nc.tensor.transpose(pt[:, i, :],
                                    vt[b][:, 128 * sk:128 * (sk + 1)],
                                    ident[:])
            for h in range(PH):
                nc.vector.tensor_copy(
                    v01[b][h][:, 4 * g4:4 * (g4 + 1), 0:64],
                    pt[:, :, 64 * h:64 * (h + 1)])

        # ---- attention machinery ----
        pending = []
        backlog = []      # heap of (ready_m, seq, thunk)
        bseq = [0]
        tail_backlog = []  # thunks drained only after the last A item
        m_count = [0]

        def backlog_push(ready, thunk):
            heapq.heappush(backlog, (ready, bseq[0], thunk))
            bseq[0] += 1
        ctx_ps = {}
        set_left = {s: 2 for s in range(NSET)}

        def emit_A(b, j, m):
            c0 = max(0, 128 * m - SC * j)
            psc = psS.tile([128, PH, SC], F32, tag="S", name=f"sc{b}_{j}_{m}")
            for h in range(PH):
                nc.tensor.matmul(
                    psc[:, h, c0:SC],
                    lhsT=kt[b][64 * h:64 * (h + 1), 128 * m:128 * (m + 1)],
                    rhs=qt[b][64 * h:64 * (h + 1), SC * j + c0:SC * (j + 1)],
                    start=True, stop=True)
            ex = exp_p.tile([128, PH, SC], BF16, tag="ex",
                            name=f"ex{b}_{j}_{m}")
            nc.scalar.activation(out=ex[:, :, c0:], in_=psc[:, :, c0:],
                                 func=EXP, scale=0.125)
            if m >= 4 * j:  # diagonal tile: zero k>q entries in the 128 block
                ea = ex[:, :, c0:c0 + 128]
                cm, eb = bass.broadcast_tensor_aps(cmask[:], ea)
                nc.vector.tensor_mul(ea, eb, cm)
            pending.append((b, j, m, ex, m == 4 * j + 3))
            m_count[0] += 1
            if len(pending) > PIPE:
                pop_av()

        def pop_av():
            b, j, m, ex, is_last = pending.pop(0)
            if m == 0:
                ctx_ps[(b, j)] = psC.tile([128, PH, SC], F32, tag="C",
                                          name=f"ctx{b}_{j}")
            cp = ctx_ps[(b, j)]
            c0 = max(0, 128 * m - SC * j)
            for h in range(PH):
                nc.tensor.matmul(
                    cp[:, h, c0:SC], lhsT=v01[b][h][:, m, :],
                    rhs=ex[:, h, c0:SC],
                    start=(m == 0), stop=is_last)
            if is_last:
                finish_chunk(b, j, cp)
                del ctx_ps[(b, j)]

        def finish_chunk(b, j, cp):
            s = 2 * b + j // 2
            # bridge replicated denominators to partitions 0..63 (ACT is
            # the only engine that can shift partitions out of PSUM)
            dn = rc_p.tile([64, PH, SC], F32, tag="dn")
            nc.scalar.copy(dn[:], cp[64:128, :, :])
            rc = rc_p.tile([64, PH, SC], F32, tag="rc")
            nc.vector.reciprocal_approx_fast(rc[:], dn[:])
            cn = cn_p.tile([64, PH, SC], BF16, tag="cn")
            nc.vector.tensor_mul(cn[:], cp[0:64, :, :], rc[:])
            for h in range(PH):
                for f in range(4):
                    # split the last batch's stores across two queues so
                    # the final pre-collective chain is short; earlier
                    # batches keep sync free for prefetches
                    eng = nc.gpsimd if (f < 2 or b < 3) else nc.sync
                    eng.dma_start(
                        out=a2a_in[s][4 * (j % 2) + f,
                                      64 * h:64 * (h + 1), :],
                        in_=cn[:, h, 128 * f:128 * (f + 1)])
            set_left[s] -= 1
            if set_left[s] == 0:
                fire_set(s)

        def fire_set(s):
            nc.gpsimd.collective_compute(
                "AllToAll", mybir.AluOpType.bypass,
                replica_groups=[list(range(NC_))],
                ins=[a2a_in[s].opt()], outs=[a2a_out[s].opt()])
            if s in (5, 6):
                # run these cats in the final drain, after the last
                # chunk's a2a_in stores and the last collective's fire
                # have been emitted: their gpsimd-blocking wait can then
                # never delay the final collective
                tail_backlog.append(_mk_cats(s))
            elif s == NSET - 1:
                backlog_push(0, _mk_cats(s))
            # cats(s-2) now: collective s-2 completed long ago (two full
            # collective periods), so its trigger never blocks the
            # gpsimd queue -- a slow collective then cannot convoy into
            # delayed a2a stores for the following sets
            if 0 <= s - 2 <= 4:
                backlog_push(0, _mk_cats(s - 2))

        def _mk_cats(s):
            def thunk():
                cats = []
                # last set: 3 parallel queues so the loads (and the
                # final outproj's first weights) land ~1us after the
                # collective completes; tail sets 5/6: sync+gpsimd (both
                # free in the drain); earlier sets: gpsimd only (a
                # blocked trigger there only delays later a2a stores)
                if s == NSET - 1:
                    engs = (nc.sync, nc.scalar, nc.gpsimd)
                elif s in (5, 6):
                    engs = (nc.sync, nc.gpsimd)
                else:
                    engs = (nc.gpsimd,)
                for r in range(NC_):
                    ct = cat_p.tile([128, 128], BF16, tag=f"cat{r}",
                                    name=f"cat{s}_{r}")
                    engs[r % len(engs)].dma_start(out=ct[:], in_=a2a_out[s][r])
                    cats.append(ct)
                for n in range(2):
                    if (s, n) in RESERVE:
                        tail_backlog.append(_mk_outproj(s, n, cats))
                    else:
                        rdy = max(m_count[0] + OUT_DELAY + 2 * n,
                                  OUT_FLOOR.get((s, n), 0))
                        backlog_push(rdy, _mk_outproj(s, n, cats))
            return thunk

        def _mk_outproj(s, n, cats):
            def thunk():
                po = psM.tile([128, SC], F32, tag="M", name=f"po{s}_{n}")
                for kp in range(ND):
                    nc.tensor.matmul(
                        po[:],
                        lhsT=cats[kp][:],
                        rhs=wot_sb[kp][:, SC * n:SC * (n + 1)],
                        start=(kp == 0), stop=(kp == ND - 1))
                ob = ob_p.tile([128, SC], F32, tag="ob")
                nc.vector.tensor_add(ob[:], po[:],
                                     bo_b[:, SC * n:SC * (n + 1)])
                r0 = 128 * s
                eng = nc.scalar if (s == NSET - 1 and n == 1) else nc.sync
                eng.dma_start(
                    out=out_shard[r0:r0 + 128, SC * n:SC * (n + 1)],
                    in_=ob[:])
            return thunk

        # ---- the merger: one global interleaved stream ----
        projW = []
        for b in range(B):
            for q in range(4):
                projW.append(("P", b, q, 0))
                if q > 0:
                    projW.append(("T", b, q - 1))
                elif b > 0:
                    projW.append(("T", b - 1, 3))
                projW.append(("P", b, q, 1))
                projW.append(("P", b, q, 2))
        projW.append(("T", 3, 3))
        attnW = [("A", b, j, m)
                 for b in range(B) for j in range(4) for m in range(4 * j + 4)]

        def cost(it):
            if it[0] == "P":
                return 4500
            if it[0] == "T":
                return 700
            _, b, j, m = it
            return 3 * (SC - max(0, 128 * m - SC * j)) + 400

        TP = sum(cost(it) for it in projW)
        TA = sum(cost(it) for it in attnW)
        # proj stream tracks attention progress plus a small lead; the
        # per-quarter readiness gate then keeps attention just behind
        # proj, so proj finishes as late as possible and the ACT-bound
        # post-proj stretch (exp is the local bottleneck) stays short
        LEAD = 0.10
        emitted = set()
        pi = ai = 0
        cp_c = ca_c = 0

        def attn_ready():
            if ai >= len(attnW):
                return False
            _, b, j, m = attnW[ai]
            qn = max(j, m // 4)
            if ("P", b, qn, 2) not in emitted:
                return False
            return ("T", b, m // 4) in emitted

        def emit_item(it):
            emitted.add(it)
            if it[0] == "P":
                emit_P(it[1], it[2], it[3])
            elif it[0] == "T":
                emit_T(it[1], it[2])
            else:
                emit_A(it[1], it[2], it[3])

        while pi < len(projW) or ai < len(attnW):
            if backlog and backlog[0][0] <= m_count[0]:
                heapq.heappop(backlog)[2]()
                continue
            ready = attn_ready()
            if pi < len(projW) and (
                    not ready or cp_c / TP < ca_c / TA + LEAD):
                cp_c += cost(projW[pi])
                emit_item(projW[pi])
                pi += 1
            elif ready:
                ca_c += cost(attnW[ai])
                emit_item(attnW[ai])
                ai += 1
            else:
                # attention gated and proj exhausted: drain backlog
                if backlog:
                    heapq.heappop(backlog)[2]()
                else:
                    raise RuntimeError("scheduler stuck")

        while pending:        # final AVs; fires the last collective
            pop_av()
        ti = 0                # reserved pieces + tail cats: PE food
        while ti < len(tail_backlog):   # (grows while iterating)
            tail_backlog[ti]()
            ti += 1
        while backlog:        # last cats + out-projection
            heapq.heappop(backlog)[2]()


_NC_CACHE = None


def _get_nc():
    global _NC_CACHE
    if _NC_CACHE is None:
        _NC_CACHE = build()
    return _NC_CACHE


def kernel(embedded, Wq, Wk, Wv, Wo, bo, _trace=False):
    import ml_dtypes
    embedded = np.asarray(embedded, np.float32)
    # emb_r[b, q, p, c, s'] = embedded[b, 512q + s', 128c + p]
    emb_r = np.ascontiguousarray(
        embedded.reshape(B, 4, SC, ND, 128).transpose(0, 1, 4, 3, 2)
    ).astype(ml_dtypes.bfloat16)
    W = np.stack([np.asarray(Wq), np.asarray(Wk), np.asarray(Wv)]).astype(
        np.float32)
    wo_t = np.ascontiguousarray(np.asarray(Wo, np.float32).T).astype(
        ml_dtypes.bfloat16).reshape(ND, 128, D)
    bo_row = np.asarray(bo, np.float32).reshape(1, D)

    in_maps = []
    for c in range(NC_):
        w = W[:, 2 * c:2 * c + 2]                  # [3, 2, D, HD]
        w = np.ascontiguousarray(w.transpose(0, 2, 1, 3)).reshape(
            3, ND, 128, 128)
        # contiguous SBUF image: [p, 3*ND chunks, 128] -> [128, 3072]
        w = np.ascontiguousarray(w.transpose(2, 0, 1, 3)).reshape(
            128, 3072).astype(ml_dtypes.bfloat16)
        in_maps.append({
            "embedded_t": emb_r,
            "w_qkv": w,
            "wo_t": wo_t,
            "bo_row": bo_row,
        })

    nc = _get_nc()
    res = run_bass_kernel_spmd(nc, in_maps, core_ids=list(range(NC_)),
                               trace=_trace)

    out = np.empty((B, S, D), np.float32)
    for c in range(NC_):
        r = res.results[c]["out_shard"]            # [1024, D]
        for s in range(8):
            b, half = s // 2, s % 2
            out[b, 1024 * half + 128 * c:1024 * half + 128 * c + 128, :] = \
                r[128 * s:128 * s + 128]
    if _trace:
        return out, res
    return out


# revision 38
# speedup vs baseline: 1.0151x; 1.0040x over previous
"""Multi-headed causal attention on 8 trn2 NeuronCores (Bass/Tile).

Sharding: tensor-parallel over heads — 2 heads per core, all 4 batches.
~335us median (baseline 421us). Design is built around three measured
facts: (1) the exp stream on ACT (~175us) exceeds attention's own PE
time, so any attention-only phase is ACT-bound; (2) every PE idle gap
>3.4us re-throttles the PE clock to 4/8 (HAM) and costs double; (3)
cores drift 10-15us apart (per-chip power throttle), so exec time is
the SLOWEST core's path, and any engine-FIFO wait on a collective can
cascade (a blocked trigger delays that queue's later a2a stores, which
delays the next collective for everyone).

  - One globally interleaved PE stream. Work items: P(b,q,p) = one
    projection accumulation group (8 matmuls [128x128]@[128x512]);
    T(b,q) = 4 PE transposes of a vt quarter (deferred one P item so
    they never wait on the vt copy); A(b,j,m) = one attention m-step
    (row-tiled concurrent score pair for the 2 heads -> exp on ACT ->
    2 AV matmuls, AVs delayed PIPE items through a pending queue);
    F(b,j) = chunk finish, triggered when the chunk's last AV pops.
  - The merger keeps proj a small LEAD ahead of attention progress with
    per-quarter readiness gating: attention runs just behind proj, exp
    work is spread over the whole kernel, and proj finishes as late as
    possible so the ACT-bound post-proj stretch stays short. The
    remaining stretch deficit is filled by floored outproj pieces.
  - Collectives: 8 half-batch AllToAlls (s = 2b + j//2, 256KB/core,
    ~4-9us data each) + a tiny warmup AllToAll at t=0. Many small sets
    resync the drifting cores often and make the final set cheap.
  - cats loads ride the gpsimd queue and are emitted only when set s+2
    fires (collective s is then long complete -> the trigger never
    actually blocks; nothing attention-critical lives on gpsimd - the
    diagonal causal mask is a DVE multiply by a precomputed mask tile).
    Sets 5/6 cats + pieces run in the final drain, after the last
    chunk's stores and the final collective's fire are emitted. The
    last set's cats split across sync/scalar/gpsimd so the final
    outproj starts ~1us after its collective completes.
  - PSUM: psS 2x[128,2,512]f32 (score pipeline), psC 1x[128,2,512]f32
    (ctx+denominators of the active chunk, single-AP finish ops), psM
    2x2KB (proj/transpose/outproj groups, each emitted atomically) =
    exactly 8 banks.
  - V is padded to [V | ones*64]: softmax denominators come out
    replicated on PSUM partitions 64..127 at zero extra PE stream time
    (out partitions are free); one ACT copy bridges them to partitions
    0..63, then reciprocal+mul on DVE.
  - Startup: ~12us of framework preamble is fixed; few large dma_starts
    (each hw queue ring holds only ~4 - excess triggers block the
    issuing engine's FIFO, which once delayed the first exp by 10us);
    dummy matmuls pre-warm the HAM clock gate while DMAs land, and a
    dummy activation pre-loads the EXP table.
  - DMA queues: bulk loads on the two hw DGE queues (sync/scalar),
    a2a_in stores on gpsimd (last batch: split gpsimd+sync), out stores
    on sync (last piece scalar).
"""
import heapq
import sys

sys.path.insert(0, "/opt/trn_rl_repo")

import numpy as np

import concourse.bass as bass
import concourse.tile as tile
from concourse import bacc, mybir
from concourse.bass_utils import run_bass_kernel_spmd

B, S, D, H, HD = 4, 2048, 1024, 16, 64
NC_ = 8          # cores
PH = 2           # heads per core
SC = 512         # s_q chunk
NK = S // 128    # 16 s_k chunks of 128
ND = D // 128    # 8 contraction chunks of 128
F32 = mybir.dt.float32
BF16 = mybir.dt.bfloat16
EXP = mybir.ActivationFunctionType.Exp
GE = mybir.AluOpType.is_ge
PIPE = 2         # AV lag behind scores, in A items
# collective sets: s = 2*b + j//2, i.e. one AllToAll per half batch
# (2 chunks, 1024 tokens, 256KB/core). Small sets keep each flight short
# (~9us), resync cores often (less skew), and make the final set cheap.
NSET = 8
# outproj pieces are floored into the ACT-bound last stage: attention
# there has a ~0.3us/m PE deficit (exp on ACT is the local bottleneck),
# and idle PE windows re-throttle the clock to 4/8. Filling them keeps
# the PE warm and does not delay the slowest core's finish (unlike
# holding pieces for the final drain, which queues ahead of the last
# outproj on the critical core).
OUT_FLOOR = {(0, 0): 126, (0, 1): 129, (1, 0): 132, (1, 1): 135,
             (2, 0): 138, (2, 1): 141, (3, 0): 144, (3, 1): 147,
             (4, 0): 150, (4, 1): 153}
OUT_DELAY = 3    # min A items after cats
# sets 5/6 resolve too late to floor safely; their cats+pieces run in
# the final drain
RESERVE = {(5, 0), (5, 1), (6, 0), (6, 1)}


def build():
    nc = bacc.Bacc("TRN2", target_bir_lowering=False, debug=False, num_devices=NC_)

    emb_t = nc.dram_tensor("embedded_t", [B, 4, 128, ND, SC], BF16,
                           kind="ExternalInput").ap()
    w_qkv = nc.dram_tensor("w_qkv", [128, 3072], BF16, kind="ExternalInput").ap()
    wo_t = nc.dram_tensor("wo_t", [ND, 128, D], BF16, kind="ExternalInput").ap()
    bo_row = nc.dram_tensor("bo_row", [1, D], F32, kind="ExternalInput").ap()
    out_shard = nc.dram_tensor("out_shard", [1024, D], F32, kind="ExternalOutput").ap()

    with tile.TileContext(nc) as tc:
        _build_body(nc, tc, emb_t, w_qkv, wo_t, bo_row, out_shard)

    nc.compile()
    return nc


def _build_body(nc, tc, emb_t, w_qkv, wo_t, bo_row, out_shard):
    from contextlib import ExitStack

    ctx = ExitStack()
    with ctx:
        const = ctx.enter_context(tc.tile_pool(name="const", bufs=1))
        psS = ctx.enter_context(tc.tile_pool(name="psS", bufs=2, space="PSUM"))
        psC = ctx.enter_context(tc.tile_pool(name="psC", bufs=1, space="PSUM"))
        psM = ctx.enter_context(tc.tile_pool(name="psM", bufs=2, space="PSUM"))
        dram = ctx.enter_context(tc.tile_pool(name="dram", bufs=1, space="DRAM"))

        etp = ctx.enter_context(tc.tile_pool(name="etp", bufs=4))
        et0p = ctx.enter_context(tc.tile_pool(name="et0p", bufs=1))
        qtp = ctx.enter_context(tc.tile_pool(name="qtp", bufs=4))
        ktp = ctx.enter_context(tc.tile_pool(name="ktp", bufs=4))
        vtp = ctx.enter_context(tc.tile_pool(name="vtp", bufs=2))
        vsb = ctx.enter_context(tc.tile_pool(name="vsb", bufs=4))
        exp_p = ctx.enter_context(tc.tile_pool(name="exp_p", bufs=6))
        rc_p = ctx.enter_context(tc.tile_pool(name="rc_p", bufs=3))
        cn_p = ctx.enter_context(tc.tile_pool(name="cn_p", bufs=6))
        cat_p = ctx.enter_context(tc.tile_pool(name="cat_p", bufs=3))
        ob_p = ctx.enter_context(tc.tile_pool(name="ob_p", bufs=3))

        # ---- warmup collective: absorb launch skew ----
        warm_in = dram.tile([NC_, 1, 16], BF16, tag="warm_in", name="warm_in")
        warm_out = dram.tile([NC_, 1, 16], BF16, tag="warm_out", name="warm_out")
        nc.gpsimd.collective_compute(
            "AllToAll", mybir.AluOpType.bypass,
            replica_groups=[list(range(NC_))],
            ins=[warm_in.opt()], outs=[warm_out.opt()])

        # ---- HAM pre-warm: the PE is idle from engine boot until the
        # first DMAs land (~15us), and would start at the cold 4/8
        # clock. Dummy matmuls (no deps, no consumers; start=True
        # overwrites the psum later) trip the activity monitor so the
        # real projection starts at full clock. They queue behind the
        # warmup collective's gpsimd rendezvous, which times their burst
        # to end right as the first weight/activation DMAs complete.
        dummy = const.tile([128, 128], BF16, tag="dummy")
        nc.gpsimd.memset(dummy[:], 0.5)
        warm_ps = psM.tile([128, SC], F32, tag="M", name="warm_ps")
        for i in range(48):
            nc.tensor.matmul(warm_ps[:, 0:128], lhsT=dummy[:], rhs=dummy[:],
                             start=True, stop=True)
        # pre-load the EXP activation table while ACT is idle, so the
        # first real exp doesn't pay the 1.3us table load
        dume = rc_p.tile([64, PH, SC], F32, tag="dn", name="dume")
        nc.scalar.activation(out=dume[:, 0, 0:128], in_=dummy[0:64, :],
                             func=EXP, scale=0.125)

        # ---- startup DMAs: only what the first matmuls need, first ----
        wq_all = const.tile([128, 24, 128], BF16, tag="wq_all")
        nc.sync.dma_start(out=wq_all[:, 0:1, :], in_=w_qkv[:, 0:128])
        nc.sync.dma_start(out=wq_all[:, 1:8, :], in_=w_qkv[:, 128:1024])
        wq_sb = [[wq_all[:, 8 * p + c, :] for c in range(ND)] for p in range(3)]

        # batch-0 j4=0: two small tiles for a fast first matmul, the rest
        # as one slab (few dma_starts: each queue only holds ~4 in its
        # ring; excess triggers block the issuing engine's FIFO)
        et0 = {}
        for c in range(2):
            t = et0p.tile([128, SC], BF16, tag=f"et0_{c}", name=f"et0_{c}")
            nc.scalar.dma_start(out=t[:], in_=emb_t[0, 0, :, c, :])
            et0[c] = t
        et0b = et0p.tile([128, ND - 2, SC], BF16, tag="et0b", name="et0b")
        nc.scalar.dma_start(out=et0b[:], in_=emb_t[0, 0, :, 2:ND, :])
        nc.sync.dma_start(out=wq_all[:, 8:24, :], in_=w_qkv[:, 1024:3072])

        quarters = {}

        def fetch_quarter(b, q):
            t = etp.tile([128, ND, SC], BF16, tag="eth", name=f"etq{b}_{q}")
            for k, eng in enumerate((nc.sync, nc.scalar)):
                eng.dma_start(out=t[:, 4 * k:4 * k + 4, :],
                              in_=emb_t[b, q, :, 4 * k:4 * k + 4, :])
            quarters[(b, q)] = t

        def et_ap(b, j4, c):
            if b == 0 and j4 == 0:
                return et0[c][:] if c < 2 else et0b[:, c - 2, :]
            return quarters[(b, j4)][:, c, :]

        fetch_quarter(0, 1)

        bo_sb = const.tile([1, D], F32, tag="bo1")
        nc.sync.dma_start(out=bo_sb[:], in_=bo_row[:])
        bo_b = const.tile([128, D], F32, tag="bob")
        nc.gpsimd.partition_broadcast(bo_b[:], bo_sb[:])

        ident = const.tile([128, 128], BF16, tag="ident")
        nc.gpsimd.memset(ident[:], 1.0)
        nc.gpsimd.affine_select(out=ident[:], in_=ident[:], compare_op=GE,
                                fill=0.0, base=0, pattern=[[-1, 128]],
                                channel_multiplier=1)
        nc.gpsimd.affine_select(out=ident[:], in_=ident[:], compare_op=GE,
                                fill=0.0, base=0, pattern=[[1, 128]],
                                channel_multiplier=-1)

        # causal mask for diagonal 128-blocks: cmask[k, 0, q] = (q >= k);
        # applied on DVE so cats loads on gpsimd never gate the AV chain
        cmask = const.tile([128, 1, 128], BF16, tag="cmask")
        nc.gpsimd.memset(cmask[:], 1.0)
        nc.gpsimd.affine_select(out=cmask[:], in_=cmask[:], compare_op=GE,
                                fill=0.0, base=0, pattern=[[0, 1], [1, 128]],
                                channel_multiplier=-1)

        wot_sb = [const.tile([128, D], BF16, tag=f"wo{c}", name=f"wo{c}")
                  for c in range(ND)]

        a2a_in = [dram.tile([NC_, 128, 128], BF16, tag=f"a2a_in{s}",
                            name=f"a2a_in{s}") for s in range(NSET)]
        a2a_out = [dram.tile([NC_, 128, 128], BF16, tag=f"a2a_out{s}",
                             name=f"a2a_out{s}") for s in range(NSET)]

        # ---- per-batch persistent tiles ----
        qt, kt, vt, v01 = {}, {}, {}, {}

        def open_batch(b):
            qt[b] = qtp.tile([128, S], BF16, tag="qt", name=f"qt{b}")
            kt[b] = ktp.tile([128, S], BF16, tag="kt", name=f"kt{b}")
            vt[b] = vtp.tile([128, S], BF16, tag="vt", name=f"vt{b}")
            v01[b] = [vsb.tile([128, NK, 128], BF16, tag=f"v{h}",
                               name=f"v{b}_{h}") for h in range(PH)]
            for h in range(PH):
                nc.vector.memset(v01[b][h][:, :, 64:128], 1.0)

        # ---- work item emitters ----
        wot_loaded = [0]
        pf_idx = [2]   # quarters 0 (et0) and 1 already fetched

        def emit_P(b, q, p):
            if p == 0:
                if b == 0 and q == 0:
                    open_batch(0)
                i = pf_idx[0]
                if i < 16:
                    fetch_quarter(i // 4, i % 4)
                    pf_idx[0] += 1
                if wot_loaded[0] < ND and b >= 1:
                    # wo isn't needed until the first outproj floor
                    # (~m=118); keep it off the early scalar queue so
                    # the exp stream never sits behind its triggers
                    c = wot_loaded[0]
                    nc.sync.dma_start(out=wot_sb[c][:], in_=wo_t[c])
                    wot_loaded[0] += 1
            ps = psM.tile([128, SC], F32, tag="M", name=f"pj{b}_{q}_{p}")
            for c in range(ND):
                nc.tensor.matmul(
                    ps[:], lhsT=wq_sb[p][c], rhs=et_ap(b, q, c),
                    start=(c == 0), stop=(c == ND - 1))
            sl = slice(SC * q, SC * (q + 1))
            if p == 0:
                nc.vector.tensor_copy(qt[b][:, sl], ps[:])
            elif p == 1:
                nc.vector.tensor_copy(kt[b][:, sl], ps[:])
            else:
                nc.vector.tensor_copy(vt[b][:, sl], ps[:])
                if q == 3 and b + 1 < B:
                    open_batch(b + 1)

        def emit_T(b, g4):
            pt = psM.tile([128, 4, 128], BF16, tag="M", name=f"tr{b}_{g4}")
            for i in range(4):
                sk = 4 * g4 + i
                nc.tensor.transpose(pt[:, i, :],
                                    vt[b][:, 128 * sk:128 * (sk + 1)],
                                    ident[:])
            for h in range(PH):
                nc.vector.tensor_copy(
                    v01[b][h][:, 4 * g4:4 * (g4 + 1), 0:64],
                    pt[:, :, 64 * h:64 * (h + 1)])

        # ---- attention machinery ----
        pending = []
        backlog = []      # heap of (ready_m, seq, thunk)
        bseq = [0]
        tail_backlog = []  # thunks drained only after the last A item
        m_count = [0]

        def backlog_push(ready, thunk):
            heapq.heappush(backlog, (ready, bseq[0], thunk))
            bseq[0] += 1
        ctx_ps = {}
        set_left = {s: 2 for s in range(NSET)}

        def emit_A(b, j, m):
            c0 = max(0, 128 * m - SC * j)
            psc = psS.tile([128, PH, SC], F32, tag="S", name=f"sc{b}_{j}_{m}")
            for h in range(PH):
                nc.tensor.matmul(
                    psc[:, h, c0:SC],
                    lhsT=kt[b][64 * h:64 * (h + 1), 128 * m:128 * (m + 1)],
                    rhs=qt[b][64 * h:64 * (h + 1), SC * j + c0:SC * (j + 1)],
                    start=True, stop=True)
            ex = exp_p.tile([128, PH, SC], BF16, tag="ex",
                            name=f"ex{b}_{j}_{m}")
            nc.scalar.activation(out=ex[:, :, c0:], in_=psc[:, :, c0:],
                                 func=EXP, scale=0.125)
            if m >= 4 * j:  # diagonal tile: zero k>q entries in the 128 block
                ea = ex[:, :, c0:c0 + 128]
                cm, eb = bass.broadcast_tensor_aps(cmask[:], ea)
                nc.vector.tensor_mul(ea, eb, cm)
            pending.append((b, j, m, ex, m == 4 * j + 3))
            m_count[0] += 1
            if len(pending) > PIPE:
                pop_av()

        def pop_av():
            b, j, m, ex, is_last = pending.pop(0)
            if m == 0:
                ctx_ps[(b, j)] = psC.tile([128, PH, SC], F32, tag="C",
                                          name=f"ctx{b}_{j}")
            cp = ctx_ps[(b, j)]
            c0 = max(0, 128 * m - SC * j)
            for h in range(PH):
                nc.tensor.matmul(
                    cp[:, h, c0:SC], lhsT=v01[b][h][:, m, :],
                    rhs=ex[:, h, c0:SC],
                    start=(m == 0), stop=is_last)
            if is_last:
                finish_chunk(b, j, cp)
                del ctx_ps[(b, j)]

        def finish_chunk(b, j, cp):
            s = 2 * b + j // 2
            # bridge replicated denominators to partitions 0..63 (ACT is
            # the only engine that can shift partitions out of PSUM)
            dn = rc_p.tile([64, PH, SC], F32, tag="dn")
            nc.scalar.copy(dn[:], cp[64:128, :, :])
            rc = rc_p.tile([64, PH, SC], F32, tag="rc")
            nc.vector.reciprocal_approx_fast(rc[:], dn[:])
            cn = cn_p.tile([64, PH, SC], BF16, tag="cn")
            nc.vector.tensor_mul(cn[:], cp[0:64, :, :], rc[:])
            for h in range(PH):
                for f in range(4):
                    # split the last batch's stores across two queues so
                    # the final pre-collective chain is short; earlier
                    # batches keep sync free for prefetches
                    eng = nc.gpsimd if (f < 2 or b < 3) else nc.sync
                    eng.dma_start(
                        out=a2a_in[s][4 * (j % 2) + f,
                                      64 * h:64 * (h + 1), :],
                        in_=cn[:, h, 128 * f:128 * (f + 1)])
            set_left[s] -= 1
            if set_left[s] == 0:
                fire_set(s)

        def fire_set(s):
            nc.gpsimd.collective_compute(
                "AllToAll", mybir.AluOpType.bypass,
                replica_groups=[list(range(NC_))],
                ins=[a2a_in[s].opt()], outs=[a2a_out[s].opt()])
            if s in (5, 6):
                # run these cats in the final drain, after the last
                # chunk's a2a_in stores and the last collective's fire
                # have been emitted: their gpsimd-blocking wait can then
                # never delay the final collective
                tail_backlog.append(_mk_cats(s))
            elif s == NSET - 1:
                backlog_push(0, _mk_cats(s))
            # cats(s-2) now: collective s-2 completed long ago (two full
            # collective periods), so its trigger never blocks the
            # gpsimd queue -- a slow collective then cannot convoy into
            # delayed a2a stores for the following sets
            if 0 <= s - 2 <= 4:
                backlog_push(0, _mk_cats(s - 2))

        def _mk_cats(s):
            def thunk():
                cats = []
                # last set: 3 parallel queues so the loads (and the
                # final outproj's first weights) land ~1us after the
                # collective completes; tail sets 5/6: sync+gpsimd (both
                # free in the drain); earlier sets: gpsimd only (a
                # blocked trigger there only delays later a2a stores)
                if s == NSET - 1:
                    engs = (nc.sync, nc.scalar, nc.gpsimd)
                elif s in (5, 6):
                    engs = (nc.sync, nc.gpsimd)
                else:
                    engs = (nc.gpsimd,)
                for r in range(NC_):
                    ct = cat_p.tile([128, 128], BF16, tag=f"cat{r}",
                                    name=f"cat{s}_{r}")
                    engs[r % len(engs)].dma_start(out=ct[:], in_=a2a_out[s][r])
                    cats.append(ct)
                for n in range(2):
                    if (s, n) in RESERVE:
                        tail_backlog.append(_mk_outproj(s, n, cats))
                    else:
                        rdy = max(m_count[0] + OUT_DELAY + 2 * n,
                                  OUT_FLOOR.get((s, n), 0))
                        backlog_push(rdy, _mk_outproj(s, n, cats))
            return thunk

        def _mk_outproj(s, n, cats):
            def thunk():
                po = psM.tile([128, SC], F32, tag="M", name=f"po{s}_{n}")
                for kp in range(ND):
                    nc.tensor.matmul(
                        po[:],
                        lhsT=cats[kp][:],
                        rhs=wot_sb[kp][:, SC * n:SC * (n + 1)],
                        start=(kp == 0), stop=(kp == ND - 1))
                ob = ob_p.tile([128, SC], F32, tag="ob")
                nc.vector.tensor_add(ob[:], po[:],
                                     bo_b[:, SC * n:SC * (n + 1)])
                r0 = 128 * s
                eng = nc.scalar if (s == NSET - 1 and n == 1) else nc.sync
                eng.dma_start(
                    out=out_shard[r0:r0 + 128, SC * n:SC * (n + 1)],
                    in_=ob[:])
            return thunk

        # ---- the merger: one global interleaved stream ----
        projW = []
        for b in range(B):
            for q in range(4):
                projW.append(("P", b, q, 0))
                if q > 0:
                    projW.append(("T", b, q - 1))
                elif b > 0:
                    projW.append(("T", b - 1, 3))
                projW.append(("P", b, q, 1))
                projW.append(("P", b, q, 2))
        projW.append(("T", 3, 3))
        attnW = [("A", b, j, m)
                 for b in range(B) for j in range(4) for m in range(4 * j + 4)]

        def cost(it):
            if it[0] == "P":
                return 4500
            if it[0] == "T":
                return 700
            _, b, j, m = it
            return 3 * (SC - max(0, 128 * m - SC * j)) + 400

        TP = sum(cost(it) for it in projW)
        TA = sum(cost(it) for it in attnW)
        # proj stream tracks attention progress plus a small lead; the
        # per-quarter readiness gate then keeps attention just behind
        # proj, so proj finishes as late as possible and the ACT-bound
        # post-proj stretch (exp is the local bottleneck) stays short
        LEAD = 0.10
        emitted = set()
        pi = ai = 0
        cp_c = ca_c = 0

        def attn_ready():
            if ai >= len(attnW):
                return False
            _, b, j, m = attnW[ai]
            qn = max(j, m // 4)
            if ("P", b, qn, 2) not in emitted:
                return False
            return ("T", b, m // 4) in emitted

        def emit_item(it):
            emitted.add(it)
            if it[0] == "P":
                emit_P(it[1], it[2], it[3])
            elif it[0] == "T":
                emit_T(it[1], it[2])
            else:
                emit_A(it[1], it[2], it[3])

        while pi < len(projW) or ai < len(attnW):
            if backlog and backlog[0][0] <= m_count[0]:
                heapq.heappop(backlog)[2]()
                continue
            ready = attn_ready()
            if pi < len(projW) and (
                    not ready or cp_c / TP < ca_c / TA + LEAD):
                cp_c += cost(projW[pi])
                emit_item(projW[pi])
                pi += 1
            elif ready:
                ca_c += cost(attnW[ai])
                emit_item(attnW[ai])
                ai += 1
            else:
                # attention gated and proj exhausted: drain backlog
                if backlog:
                    heapq.heappop(backlog)[2]()
                else:
                    raise RuntimeError("scheduler stuck")

        while pending:        # final AVs; fires the last collective
            pop_av()
        ti = 0                # reserved pieces + tail cats: PE food
        while ti < len(tail_backlog):   # (grows while iterating)
            tail_backlog[ti]()
            ti += 1
        while backlog:        # last cats + out-projection
            heapq.heappop(backlog)[2]()


_NC_CACHE = None


def _get_nc():
    global _NC_CACHE
    if _NC_CACHE is None:
        _NC_CACHE = build()
    return _NC_CACHE


def kernel(embedded, Wq, Wk, Wv, Wo, bo, _trace=False):
    import ml_dtypes
    embedded = np.asarray(embedded, np.float32)
    # emb_r[b, q, p, c, s'] = embedded[b, 512q + s', 128c + p]
    emb_r = np.ascontiguousarray(
        embedded.reshape(B, 4, SC, ND, 128).transpose(0, 1, 4, 3, 2)
    ).astype(ml_dtypes.bfloat16)
    W = np.stack([np.asarray(Wq), np.asarray(Wk), np.asarray(Wv)]).astype(
        np.float32)
    wo_t = np.ascontiguousarray(np.asarray(Wo, np.float32).T).astype(
        ml_dtypes.bfloat16).reshape(ND, 128, D)
    bo_row = np.asarray(bo, np.float32).reshape(1, D)

    in_maps = []
    for c in range(NC_):
        w = W[:, 2 * c:2 * c + 2]                  # [3, 2, D, HD]
        w = np.ascontiguousarray(w.transpose(0, 2, 1, 3)).reshape(
            3, ND, 128, 128)
        # contiguous SBUF image: [p, 3*ND chunks, 128] -> [128, 3072]
        w = np.ascontiguousarray(w.transpose(2, 0, 1, 3)).reshape(
            128, 3072).astype(ml_dtypes.bfloat16)
        in_maps.append({
            "embedded_t": emb_r,
            "w_qkv": w,
            "wo_t": wo_t,
            "bo_row": bo_row,
        })

    nc = _get_nc()
    res = run_bass_kernel_spmd(nc, in_maps, core_ids=list(range(NC_)),
                               trace=_trace)

    out = np.empty((B, S, D), np.float32)
    for c in range(NC_):
        r = res.results[c]["out_shard"]            # [1024, D]
        for s in range(8):
            b, half = s // 2, s % 2
            out[b, 1024 * half + 128 * c:1024 * half + 128 * c + 128, :] = \
                r[128 * s:128 * s + 128]
    if _trace:
        return out, res
    return out


# revision 39
# speedup vs baseline: 1.0202x; 1.0050x over previous
"""Multi-headed causal attention on 8 trn2 NeuronCores (Bass/Tile).

Sharding: tensor-parallel over heads — 2 heads per core, all 4 batches.
~335us median (baseline 421us). Design is built around three measured
facts: (1) the exp stream on ACT (~175us) exceeds attention's own PE
time, so any attention-only phase is ACT-bound; (2) every PE idle gap
>3.4us re-throttles the PE clock to 4/8 (HAM) and costs double; (3)
cores drift 10-15us apart (per-chip power throttle), so exec time is
the SLOWEST core's path, and any engine-FIFO wait on a collective can
cascade (a blocked trigger delays that queue's later a2a stores, which
delays the next collective for everyone).

  - One globally interleaved PE stream. Work items: P(b,q,p) = one
    projection accumulation group (8 matmuls [128x128]@[128x512]);
    T(b,q) = 4 PE transposes of a vt quarter (deferred one P item so
    they never wait on the vt copy); A(b,j,m) = one attention m-step
    (row-tiled concurrent score pair for the 2 heads -> exp on ACT ->
    2 AV matmuls, AVs delayed PIPE items through a pending queue);
    F(b,j) = chunk finish, triggered when the chunk's last AV pops.
  - The merger keeps proj a small LEAD ahead of attention progress with
    per-quarter readiness gating: attention runs just behind proj, exp
    work is spread over the whole kernel, and proj finishes as late as
    possible so the ACT-bound post-proj stretch stays short. The
    remaining stretch deficit is filled by floored outproj pieces.
  - Collectives: 8 half-batch AllToAlls (s = 2b + j//2, 256KB/core,
    ~4-9us data each) + a tiny warmup AllToAll at t=0. Many small sets
    resync the drifting cores often and make the final set cheap.
  - cats loads ride the gpsimd queue and are emitted only when set s+2
    fires (collective s is then long complete -> the trigger never
    actually blocks; nothing attention-critical lives on gpsimd - the
    diagonal causal mask is a DVE multiply by a precomputed mask tile).
    Sets 5/6 cats + pieces run in the final drain, after the last
    chunk's stores and the final collective's fire are emitted. The
    last set's cats split across sync/scalar/gpsimd so the final
    outproj starts ~1us after its collective completes.
  - PSUM: psS 2x[128,2,512]f32 (score pipeline), psC 1x[128,2,512]f32
    (ctx+denominators of the active chunk, single-AP finish ops), psM
    2x2KB (proj/transpose/outproj groups, each emitted atomically) =
    exactly 8 banks.
  - V is padded to [V | ones*64]: softmax denominators come out
    replicated on PSUM partitions 64..127 at zero extra PE stream time
    (out partitions are free); one ACT copy bridges them to partitions
    0..63, then reciprocal+mul on DVE.
  - Startup: ~12us of framework preamble is fixed; few large dma_starts
    (each hw queue ring holds only ~4 - excess triggers block the
    issuing engine's FIFO, which once delayed the first exp by 10us);
    dummy matmuls pre-warm the HAM clock gate while DMAs land, and a
    dummy activation pre-loads the EXP table.
  - DMA queues: bulk loads on the two hw DGE queues (sync/scalar),
    a2a_in stores on gpsimd (last batch: split gpsimd+sync), out stores
    on sync (last piece scalar).
"""
import heapq
import sys

sys.path.insert(0, "/opt/trn_rl_repo")

import numpy as np

import concourse.bass as bass
import concourse.tile as tile
from concourse import bacc, mybir
from concourse.bass_utils import run_bass_kernel_spmd

B, S, D, H, HD = 4, 2048, 1024, 16, 64
NC_ = 8          # cores
PH = 2           # heads per core
SC = 512         # s_q chunk
NK = S // 128    # 16 s_k chunks of 128
ND = D // 128    # 8 contraction chunks of 128
F32 = mybir.dt.float32
BF16 = mybir.dt.bfloat16
EXP = mybir.ActivationFunctionType.Exp
GE = mybir.AluOpType.is_ge
PIPE = 2         # AV lag behind scores, in A items
# collective sets: s = 2*b + j//2, i.e. one AllToAll per half batch
# (2 chunks, 1024 tokens, 256KB/core). Small sets keep each flight short
# (~9us), resync cores often (less skew), and make the final set cheap.
NSET = 8
# outproj pieces are floored into the ACT-bound last stage: attention
# there has a ~0.3us/m PE deficit (exp on ACT is the local bottleneck),
# and idle PE windows re-throttle the clock to 4/8. Filling them keeps
# the PE warm and does not delay the slowest core's finish (unlike
# holding pieces for the final drain, which queues ahead of the last
# outproj on the critical core).
OUT_FLOOR = {(0, 0): 126, (0, 1): 129, (1, 0): 132, (1, 1): 135,
             (2, 0): 138, (2, 1): 141, (3, 0): 144, (3, 1): 147,
             (4, 0): 150, (4, 1): 153}
OUT_DELAY = 3    # min A items after cats
# sets 5/6 resolve too late to floor safely; their cats+pieces run in
# the final drain
RESERVE = {(5, 0), (5, 1), (6, 0), (6, 1)}


def build():
    nc = bacc.Bacc("TRN2", target_bir_lowering=False, debug=False, num_devices=NC_)

    emb_t = nc.dram_tensor("embedded_t", [B, 4, 128, ND, SC], BF16,
                           kind="ExternalInput").ap()
    w_qkv = nc.dram_tensor("w_qkv", [128, 3072], BF16, kind="ExternalInput").ap()
    wo_t = nc.dram_tensor("wo_t", [ND, 128, D], BF16, kind="ExternalInput").ap()
    bo_row = nc.dram_tensor("bo_row", [1, D], F32, kind="ExternalInput").ap()
    out_shard = nc.dram_tensor("out_shard", [1024, D], F32, kind="ExternalOutput").ap()

    with tile.TileContext(nc) as tc:
        _build_body(nc, tc, emb_t, w_qkv, wo_t, bo_row, out_shard)

    nc.compile()
    return nc


def _build_body(nc, tc, emb_t, w_qkv, wo_t, bo_row, out_shard):
    from contextlib import ExitStack

    ctx = ExitStack()
    with ctx:
        const = ctx.enter_context(tc.tile_pool(name="const", bufs=1))
        psS = ctx.enter_context(tc.tile_pool(name="psS", bufs=2, space="PSUM"))
        psC = ctx.enter_context(tc.tile_pool(name="psC", bufs=1, space="PSUM"))
        psM = ctx.enter_context(tc.tile_pool(name="psM", bufs=2, space="PSUM"))
        dram = ctx.enter_context(tc.tile_pool(name="dram", bufs=1, space="DRAM"))

        etp = ctx.enter_context(tc.tile_pool(name="etp", bufs=4))
        et0p = ctx.enter_context(tc.tile_pool(name="et0p", bufs=1))
        qtp = ctx.enter_context(tc.tile_pool(name="qtp", bufs=4))
        ktp = ctx.enter_context(tc.tile_pool(name="ktp", bufs=4))
        vtp = ctx.enter_context(tc.tile_pool(name="vtp", bufs=2))
        vsb = ctx.enter_context(tc.tile_pool(name="vsb", bufs=4))
        exp_p = ctx.enter_context(tc.tile_pool(name="exp_p", bufs=6))
        rc_p = ctx.enter_context(tc.tile_pool(name="rc_p", bufs=3))
        cn_p = ctx.enter_context(tc.tile_pool(name="cn_p", bufs=6))
        cat_p = ctx.enter_context(tc.tile_pool(name="cat_p", bufs=3))
        ob_p = ctx.enter_context(tc.tile_pool(name="ob_p", bufs=3))

        # ---- warmup collective: absorb launch skew ----
        warm_in = dram.tile([NC_, 1, 16], BF16, tag="warm_in", name="warm_in")
        warm_out = dram.tile([NC_, 1, 16], BF16, tag="warm_out", name="warm_out")
        nc.gpsimd.collective_compute(
            "AllToAll", mybir.AluOpType.bypass,
            replica_groups=[list(range(NC_))],
            ins=[warm_in.opt()], outs=[warm_out.opt()])

        # ---- HAM pre-warm: the PE is idle from engine boot until the
        # first DMAs land (~15us), and would start at the cold 4/8
        # clock. Dummy matmuls (no deps, no consumers; start=True
        # overwrites the psum later) trip the activity monitor so the
        # real projection starts at full clock. They queue behind the
        # warmup collective's gpsimd rendezvous, which times their burst
        # to end right as the first weight/activation DMAs complete.
        dummy = const.tile([128, 128], BF16, tag="dummy")
        nc.gpsimd.memset(dummy[:], 0.5)
        warm_ps = psM.tile([128, SC], F32, tag="M", name="warm_ps")
        for i in range(48):
            nc.tensor.matmul(warm_ps[:, 0:128], lhsT=dummy[:], rhs=dummy[:],
                             start=True, stop=True)
        # pre-load the EXP activation table while ACT is idle, so the
        # first real exp doesn't pay the 1.3us table load
        dume = rc_p.tile([64, PH, SC], F32, tag="dn", name="dume")
        nc.scalar.activation(out=dume[:, 0, 0:128], in_=dummy[0:64, :],
                             func=EXP, scale=0.125)

        # ---- startup DMAs: only what the first matmuls need, first ----
        wq_all = const.tile([128, 24, 128], BF16, tag="wq_all")
        nc.sync.dma_start(out=wq_all[:, 0:1, :], in_=w_qkv[:, 0:128])
        nc.sync.dma_start(out=wq_all[:, 1:8, :], in_=w_qkv[:, 128:1024])
        wq_sb = [[wq_all[:, 8 * p + c, :] for c in range(ND)] for p in range(3)]

        # batch-0 j4=0: two small tiles for a fast first matmul, the rest
        # as one slab (few dma_starts: each queue only holds ~4 in its
        # ring; excess triggers block the issuing engine's FIFO)
        et0 = {}
        for c in range(2):
            t = et0p.tile([128, SC], BF16, tag=f"et0_{c}", name=f"et0_{c}")
            nc.scalar.dma_start(out=t[:], in_=emb_t[0, 0, :, c, :])
            et0[c] = t
        # two halves so the first projection group can stream: a single
        # slab would gate chunk c=2 on the whole 768KB landing
        et0b = et0p.tile([128, ND - 2, SC], BF16, tag="et0b", name="et0b")
        nc.scalar.dma_start(out=et0b[:, 0:3, :], in_=emb_t[0, 0, :, 2:5, :])
        nc.scalar.dma_start(out=et0b[:, 3:6, :], in_=emb_t[0, 0, :, 5:ND, :])
        nc.sync.dma_start(out=wq_all[:, 8:24, :], in_=w_qkv[:, 1024:3072])

        quarters = {}

        def fetch_quarter(b, q):
            t = etp.tile([128, ND, SC], BF16, tag="eth", name=f"etq{b}_{q}")
            for k, eng in enumerate((nc.sync, nc.scalar)):
                eng.dma_start(out=t[:, 4 * k:4 * k + 4, :],
                              in_=emb_t[b, q, :, 4 * k:4 * k + 4, :])
            quarters[(b, q)] = t

        def et_ap(b, j4, c):
            if b == 0 and j4 == 0:
                return et0[c][:] if c < 2 else et0b[:, c - 2, :]
            return quarters[(b, j4)][:, c, :]

        fetch_quarter(0, 1)

        bo_sb = const.tile([1, D], F32, tag="bo1")
        nc.sync.dma_start(out=bo_sb[:], in_=bo_row[:])
        bo_b = const.tile([128, D], F32, tag="bob")
        nc.gpsimd.partition_broadcast(bo_b[:], bo_sb[:])

        ident = const.tile([128, 128], BF16, tag="ident")
        nc.gpsimd.memset(ident[:], 1.0)
        nc.gpsimd.affine_select(out=ident[:], in_=ident[:], compare_op=GE,
                                fill=0.0, base=0, pattern=[[-1, 128]],
                                channel_multiplier=1)
        nc.gpsimd.affine_select(out=ident[:], in_=ident[:], compare_op=GE,
                                fill=0.0, base=0, pattern=[[1, 128]],
                                channel_multiplier=-1)

        # causal mask for diagonal 128-blocks: cmask[k, 0, q] = (q >= k);
        # applied on DVE so cats loads on gpsimd never gate the AV chain
        cmask = const.tile([128, 1, 128], BF16, tag="cmask")
        nc.gpsimd.memset(cmask[:], 1.0)
        nc.gpsimd.affine_select(out=cmask[:], in_=cmask[:], compare_op=GE,
                                fill=0.0, base=0, pattern=[[0, 1], [1, 128]],
                                channel_multiplier=-1)

        wot_sb = [const.tile([128, D], BF16, tag=f"wo{c}", name=f"wo{c}")
                  for c in range(ND)]

        a2a_in = [dram.tile([NC_, 128, 128], BF16, tag=f"a2a_in{s}",
                            name=f"a2a_in{s}") for s in range(NSET)]
        a2a_out = [dram.tile([NC_, 128, 128], BF16, tag=f"a2a_out{s}",
                             name=f"a2a_out{s}") for s in range(NSET)]

        # ---- per-batch persistent tiles ----
        qt, kt, vt, v01 = {}, {}, {}, {}

        def open_batch(b):
            qt[b] = qtp.tile([128, S], BF16, tag="qt", name=f"qt{b}")
            kt[b] = ktp.tile([128, S], BF16, tag="kt", name=f"kt{b}")
            vt[b] = vtp.tile([128, S], BF16, tag="vt", name=f"vt{b}")
            v01[b] = [vsb.tile([128, NK, 128], BF16, tag=f"v{h}",
                               name=f"v{b}_{h}") for h in range(PH)]
            for h in range(PH):
                nc.vector.memset(v01[b][h][:, :, 64:128], 1.0)

        # ---- work item emitters ----
        wot_loaded = [0]
        pf_idx = [2]   # quarters 0 (et0) and 1 already fetched

        def emit_P(b, q, p):
            if p == 0:
                if b == 0 and q == 0:
                    open_batch(0)
                i = pf_idx[0]
                if i < 16:
                    fetch_quarter(i // 4, i % 4)
                    pf_idx[0] += 1
                if wot_loaded[0] < ND and b >= 1:
                    # wo isn't needed until the first outproj floor
                    # (~m=118); keep it off the early scalar queue so
                    # the exp stream never sits behind its triggers
                    c = wot_loaded[0]
                    nc.sync.dma_start(out=wot_sb[c][:], in_=wo_t[c])
                    wot_loaded[0] += 1
            ps = psM.tile([128, SC], F32, tag="M", name=f"pj{b}_{q}_{p}")
            for c in range(ND):
                nc.tensor.matmul(
                    ps[:], lhsT=wq_sb[p][c], rhs=et_ap(b, q, c),
                    start=(c == 0), stop=(c == ND - 1))
            sl = slice(SC * q, SC * (q + 1))
            if p == 0:
                nc.vector.tensor_copy(qt[b][:, sl], ps[:])
            elif p == 1:
                nc.vector.tensor_copy(kt[b][:, sl], ps[:])
            else:
                nc.vector.tensor_copy(vt[b][:, sl], ps[:])
                if q == 3 and b + 1 < B:
                    open_batch(b + 1)

        def emit_T(b, g4):
            pt = psM.tile([128, 4, 128], BF16, tag="M", name=f"tr{b}_{g4}")
            for i in range(4):
                sk = 4 * g4 + i
                nc.tensor.transpose(pt[:, i, :],
                                    vt[b][:, 128 * sk:128 * (sk + 1)],
                                    ident[:])
            for h in range(PH):
                nc.vector.tensor_copy(
                    v01[b][h][:, 4 * g4:4 * (g4 + 1), 0:64],
                    pt[:, :, 64 * h:64 * (h + 1)])

        # ---- attention machinery ----
        pending = []
        backlog = []      # heap of (ready_m, seq, thunk)
        bseq = [0]
        tail_backlog = []  # thunks drained only after the last A item
        m_count = [0]

        def backlog_push(ready, thunk):
            heapq.heappush(backlog, (ready, bseq[0], thunk))
            bseq[0] += 1
        ctx_ps = {}
        set_left = {s: 2 for s in range(NSET)}

        def emit_A(b, j, m):
            c0 = max(0, 128 * m - SC * j)
            psc = psS.tile([128, PH, SC], F32, tag="S", name=f"sc{b}_{j}_{m}")
            for h in range(PH):
                nc.tensor.matmul(
                    psc[:, h, c0:SC],
                    lhsT=kt[b][64 * h:64 * (h + 1), 128 * m:128 * (m + 1)],
                    rhs=qt[b][64 * h:64 * (h + 1), SC * j + c0:SC * (j + 1)],
                    start=True, stop=True)
            ex = exp_p.tile([128, PH, SC], BF16, tag="ex",
                            name=f"ex{b}_{j}_{m}")
            nc.scalar.activation(out=ex[:, :, c0:], in_=psc[:, :, c0:],
                                 func=EXP, scale=0.125)
            if m >= 4 * j:  # diagonal tile: zero k>q entries in the 128 block
                ea = ex[:, :, c0:c0 + 128]
                cm, eb = bass.broadcast_tensor_aps(cmask[:], ea)
                nc.vector.tensor_mul(ea, eb, cm)
            pending.append((b, j, m, ex, m == 4 * j + 3))
            m_count[0] += 1
            if len(pending) > PIPE:
                pop_av()

        def pop_av():
            b, j, m, ex, is_last = pending.pop(0)
            if m == 0:
                ctx_ps[(b, j)] = psC.tile([128, PH, SC], F32, tag="C",
                                          name=f"ctx{b}_{j}")
            cp = ctx_ps[(b, j)]
            c0 = max(0, 128 * m - SC * j)
            for h in range(PH):
                nc.tensor.matmul(
                    cp[:, h, c0:SC], lhsT=v01[b][h][:, m, :],
                    rhs=ex[:, h, c0:SC],
                    start=(m == 0), stop=is_last)
            if is_last:
                finish_chunk(b, j, cp)
                del ctx_ps[(b, j)]

        def finish_chunk(b, j, cp):
            s = 2 * b + j // 2
            # bridge replicated denominators to partitions 0..63 (ACT is
            # the only engine that can shift partitions out of PSUM)
            dn = rc_p.tile([64, PH, SC], F32, tag="dn")
            nc.scalar.copy(dn[:], cp[64:128, :, :])
            rc = rc_p.tile([64, PH, SC], F32, tag="rc")
            nc.vector.reciprocal_approx_fast(rc[:], dn[:])
            cn = cn_p.tile([64, PH, SC], BF16, tag="cn")
            nc.vector.tensor_mul(cn[:], cp[0:64, :, :], rc[:])
            for h in range(PH):
                for f in range(4):
                    # split the last batch's stores across two queues so
                    # the final pre-collective chain is short; earlier
                    # batches keep sync free for prefetches
                    eng = nc.gpsimd if (f < 2 or b < 3) else nc.sync
                    eng.dma_start(
                        out=a2a_in[s][4 * (j % 2) + f,
                                      64 * h:64 * (h + 1), :],
                        in_=cn[:, h, 128 * f:128 * (f + 1)])
            set_left[s] -= 1
            if set_left[s] == 0:
                fire_set(s)

        def fire_set(s):
            nc.gpsimd.collective_compute(
                "AllToAll", mybir.AluOpType.bypass,
                replica_groups=[list(range(NC_))],
                ins=[a2a_in[s].opt()], outs=[a2a_out[s].opt()])
            if s in (5, 6):
                # run these cats in the final drain, after the last
                # chunk's a2a_in stores and the last collective's fire
                # have been emitted: their gpsimd-blocking wait can then
                # never delay the final collective
                tail_backlog.append(_mk_cats(s))
            elif s == NSET - 1:
                backlog_push(0, _mk_cats(s))
            # cats(s-2) now: collective s-2 completed long ago (two full
            # collective periods), so its trigger never blocks the
            # gpsimd queue -- a slow collective then cannot convoy into
            # delayed a2a stores for the following sets
            if 0 <= s - 2 <= 4:
                backlog_push(0, _mk_cats(s - 2))

        def _mk_cats(s):
            def thunk():
                cats = []
                # last set: 3 parallel queues so the loads (and the
                # final outproj's first weights) land ~1us after the
                # collective completes; tail sets 5/6: sync+gpsimd (both
                # free in the drain); earlier sets: gpsimd only (a
                # blocked trigger there only delays later a2a stores)
                if s == NSET - 1:
                    engs = (nc.sync, nc.scalar, nc.gpsimd)
                elif s in (5, 6):
                    engs = (nc.sync, nc.gpsimd)
                else:
                    engs = (nc.gpsimd,)
                for r in range(NC_):
                    ct = cat_p.tile([128, 128], BF16, tag=f"cat{r}",
                                    name=f"cat{s}_{r}")
                    engs[r % len(engs)].dma_start(out=ct[:], in_=a2a_out[s][r])
                    cats.append(ct)
                for n in range(2):
                    if (s, n) in RESERVE:
                        tail_backlog.append(_mk_outproj(s, n, cats))
                    else:
                        rdy = max(m_count[0] + OUT_DELAY + 2 * n,
                                  OUT_FLOOR.get((s, n), 0))
                        backlog_push(rdy, _mk_outproj(s, n, cats))
            return thunk

        def _mk_outproj(s, n, cats):
            def thunk():
                po = psM.tile([128, SC], F32, tag="M", name=f"po{s}_{n}")
                for kp in range(ND):
                    nc.tensor.matmul(
                        po[:],
                        lhsT=cats[kp][:],
                        rhs=wot_sb[kp][:, SC * n:SC * (n + 1)],
                        start=(kp == 0), stop=(kp == ND - 1))
                ob = ob_p.tile([128, SC], F32, tag="ob")
                nc.vector.tensor_add(ob[:], po[:],
                                     bo_b[:, SC * n:SC * (n + 1)])
                r0 = 128 * s
                eng = nc.scalar if (s == NSET - 1 and n == 1) else nc.sync
                eng.dma_start(
                    out=out_shard[r0:r0 + 128, SC * n:SC * (n + 1)],
                    in_=ob[:])
            return thunk

        # ---- the merger: one global interleaved stream ----
        projW = []
        for b in range(B):
            for q in range(4):
                projW.append(("P", b, q, 0))
                if q > 0:
                    projW.append(("T", b, q - 1))
                elif b > 0:
                    projW.append(("T", b - 1, 3))
                projW.append(("P", b, q, 1))
                projW.append(("P", b, q, 2))
        projW.append(("T", 3, 3))
        attnW = [("A", b, j, m)
                 for b in range(B) for j in range(4) for m in range(4 * j + 4)]

        def cost(it):
            if it[0] == "P":
                return 4500
            if it[0] == "T":
                return 700
            _, b, j, m = it
            return 3 * (SC - max(0, 128 * m - SC * j)) + 400

        TP = sum(cost(it) for it in projW)
        TA = sum(cost(it) for it in attnW)
        # proj stream tracks attention progress plus a small lead; the
        # per-quarter readiness gate then keeps attention just behind
        # proj, so proj finishes as late as possible and the ACT-bound
        # post-proj stretch (exp is the local bottleneck) stays short
        LEAD = 0.10
        emitted = set()
        pi = ai = 0
        cp_c = ca_c = 0

        def attn_ready():
            if ai >= len(attnW):
                return False
            _, b, j, m = attnW[ai]
            qn = max(j, m // 4)
            if ("P", b, qn, 2) not in emitted:
                return False
            return ("T", b, m // 4) in emitted

        def emit_item(it):
            emitted.add(it)
            if it[0] == "P":
                emit_P(it[1], it[2], it[3])
            elif it[0] == "T":
                emit_T(it[1], it[2])
            else:
                emit_A(it[1], it[2], it[3])

        while pi < len(projW) or ai < len(attnW):
            if backlog and backlog[0][0] <= m_count[0]:
                heapq.heappop(backlog)[2]()
                continue
            ready = attn_ready()
            if pi < len(projW) and (
                    not ready or cp_c / TP < ca_c / TA + LEAD):
                cp_c += cost(projW[pi])
                emit_item(projW[pi])
                pi += 1
            elif ready:
                ca_c += cost(attnW[ai])
                emit_item(attnW[ai])
                ai += 1
            else:
                # attention gated and proj exhausted: drain backlog
                if backlog:
                    heapq.heappop(backlog)[2]()
                else:
                    raise RuntimeError("scheduler stuck")

        while pending:        # final AVs; fires the last collective
            pop_av()
        ti = 0                # reserved pieces + tail cats: PE food
        while ti < len(tail_backlog):   # (grows while iterating)
            tail_backlog[ti]()
            ti += 1
        while backlog:        # last cats + out-projection
            heapq.heappop(backlog)[2]()


_NC_CACHE = None


def _get_nc():
    global _NC_CACHE
    if _NC_CACHE is None:
        _NC_CACHE = build()
    return _NC_CACHE


def kernel(embedded, Wq, Wk, Wv, Wo, bo, _trace=False):
    import ml_dtypes
    embedded = np.asarray(embedded, np.float32)
    # emb_r[b, q, p, c, s'] = embedded[b, 512q + s', 128c + p]
    emb_r = np.ascontiguousarray(
        embedded.reshape(B, 4, SC, ND, 128).transpose(0, 1, 4, 3, 2)
    ).astype(ml_dtypes.bfloat16)
    W = np.stack([np.asarray(Wq), np.asarray(Wk), np.asarray(Wv)]).astype(
        np.float32)
    wo_t = np.ascontiguousarray(np.asarray(Wo, np.float32).T).astype(
        ml_dtypes.bfloat16).reshape(ND, 128, D)
    bo_row = np.asarray(bo, np.float32).reshape(1, D)

    in_maps = []
    for c in range(NC_):
        w = W[:, 2 * c:2 * c + 2]                  # [3, 2, D, HD]
        w = np.ascontiguousarray(w.transpose(0, 2, 1, 3)).reshape(
            3, ND, 128, 128)
        # contiguous SBUF image: [p, 3*ND chunks, 128] -> [128, 3072]
        w = np.ascontiguousarray(w.transpose(2, 0, 1, 3)).reshape(
            128, 3072).astype(ml_dtypes.bfloat16)
        in_maps.append({
            "embedded_t": emb_r,
            "w_qkv": w,
            "wo_t": wo_t,
            "bo_row": bo_row,
        })

    nc = _get_nc()
    res = run_bass_kernel_spmd(nc, in_maps, core_ids=list(range(NC_)),
                               trace=_trace)

    out = np.empty((B, S, D), np.float32)
    for c in range(NC_):
        r = res.results[c]["out_shard"]            # [1024, D]
        for s in range(8):
            b, half = s // 2, s % 2
            out[b, 1024 * half + 128 * c:1024 * half + 128 * c + 128, :] = \
                r[128 * s:128 * s + 128]
    if _trace:
        return out, res
    return out
